# revision 36
# baseline (speedup 1.0000x reference)
"""Trainium2 Bass kernel for nn_Graph_module_net_0_loss_type_18631568130084.

GNN message-passing block:
  gts       = relu(gt_feat @ Wg + bg)
  attn[i,j] = sigmoid(x[j]@Wq + x[i]@Wk + b_att)          (H == 1)
  atten     = (attn * (mr1+mr2) * col + f_diag) / CHILDS  ([B,H,Nj,Ni])
  o1 = relu(gconv1(x^T)); o1 += ln1(o1 @ atten)^T
  o2 = relu(gconv2(o1));  node_feat = ln2(o2 @ atten);  output2 = (o2 + node_feat^T)^T

Sharding: data-parallel over batch B=16 -> 2 batches per core on 8 cores.

v3 design notes (on top of the v1 transposed layout):
 * Masks pre-combined on host: m12c = (m1+m2) * (score*col)[j] in fp8e4
   ({0,1,2} and the zeros are exact).  The f_diag diagonal is folded in as
   fd[i]/sigmoid(z_ii) (exact where score==0 because the mask diag is 0
   there), so atten^T = m12c ⊙ sigmoid(z) with no device-side fixups.
 * The two big [N,N]x[N,M] contractions (stages D/F) run as fp8e4 DoubleRow
   matmuls (2 j-chunks per instruction).
 * fp16->fp8 casts of o1/o2 and the o1 residual add ride the DMA engines
   (SWDGE cast-DMA / accum-add DMA), not the compute engines.
 * ln2's gamma/beta are applied on the HOST (node_feat = xhat2*g2+b2 and
   output2 = o2 + node_feat), so stage F only standardizes.
 * Grouped convs use packed per-chunk weights (no block-diag zero columns).
 * Inputs/outputs move as one large DMA per tensor per batch.
 * The global 1/CHILDS scale cancels inside both layernorms, so it is dropped
   and eps is rescaled by CHILDS^2 to keep the math exactly equivalent.
 * The top-k "col" mask is computed exactly on the host: a cheap sufficient
   condition proves col == all-ones; otherwise an exact numpy replica runs.
"""

import numpy as np
import ml_dtypes

B = 16
N = 1024
CIN = 256
MID = 512
OUT = 256
G = 4
CHILDS = 512
NCORES = 8
B_LOC = B // NCORES  # 2
NT = N // 128  # 8
EPS_LN = 1e-6 * float(CHILDS) ** 2  # eps rescaled because we drop the 1/CHILDS

F16 = np.float16
F32 = np.float32
F8 = ml_dtypes.float8_e4m3

_PROGRAM_CACHE = {}


def _build_program(beta1_nz: bool):
    import concourse.bacc as bacc
    import concourse.bass as bass
    import concourse.tile as tile
    from concourse import mybir

    f8 = mybir.dt.float8e4
    f16 = mybir.dt.float16
    bf16 = mybir.dt.bfloat16
    f32 = mybir.dt.float32
    AF = mybir.ActivationFunctionType
    OP = mybir.AluOpType
    DR = mybir.MatmulPerfMode.DoubleRow

    nc = bacc.Bacc("TRN2", debug=False)

    def din(name, shape, dt):
        return nc.dram_tensor(name, shape, dt, kind="ExternalInput").ap()

    def dout(name, shape, dt):
        return nc.dram_tensor(name, shape, dt, kind="ExternalOutput").ap()

    # Per-core inputs (leading dim B_LOC where batch-dependent).
    m12_d = din("m12cT", [B_LOC, N, N], f8)       # (m1+m2)*(score*col) + diag, ^T
    gxT_d = din("gxT", [B_LOC, 2 * CIN, N], f16)  # [gt^T ; x^T]  [c, n]
    lirow_d = din("lirow", [B_LOC, N], f16)       # x@Wk + b_att      (per-i row)
    ljT_d = din("ljT", [B_LOC, 128, NT], f32)     # x@Wq chunked      (per-j bias)
    # Replicated weights, packed: blk = [ident | wg 2x256 | w1 2x256 | w2 4x64],
    # crow = [ones 128 | bg 256 | b1 512 | b2 256].
    blk_d = din("constblk", [128, 1408], f16)
    crow_d = din("constrow", [1, 1152], f16)
    g1_d = din("g1row", [1, MID], bf16)
    beta1_d = din("beta1row", [1, MID], f32)

    gts_d = dout("gts", [B_LOC, N, OUT], f16)
    node_d = dout("node", [B_LOC, N, OUT], f16)   # standardized xhat2 (pre-g2)
    o2o_d = dout("o2t", [B_LOC, N, OUT], f16)

    with tile.TileContext(nc) as tc:
        with tc.tile_pool(name="const", bufs=1) as constp, \
             tc.tile_pool(name="big", bufs=2) as bigp, \
             tc.tile_pool(name="work", bufs=4) as workp, \
             tc.tile_pool(name="outs", bufs=3) as outp, \
             tc.tile_pool(name="mm", bufs=2, space="PSUM") as mmp, \
             tc.tile_pool(name="tp", bufs=2, space="PSUM") as tpp:

            # ---- early DMAs: masks on the SWDGE queue, sigmoid inputs on
            # scalar, conv inputs + packed consts on sync ----
            # tiny sigmoid inputs FIRST (the big mask DMAs would otherwise
            # head-of-line-block them on the DMA engines)
            lirow_ts, ljT_ts = [], []
            for b in range(B_LOC):
                lirow_t = workp.tile([128, N], f16, tag="lirow")
                nc.scalar.dma_start(
                    out=lirow_t, in_=lirow_d[b : b + 1, :].to_broadcast([128, N])
                )
                lirow_ts.append(lirow_t)
                ljT_t = workp.tile([128, NT], f32, tag="ljT")
                nc.scalar.dma_start(out=ljT_t, in_=ljT_d[b])
                ljT_ts.append(ljT_t)
            m12b_t = bigp.tile([128, B_LOC, NT, N], f8, name="m12b", tag="m12b")
            for b in range(B_LOC):
                for h in range(2):
                    nc.gpsimd.dma_start(
                        out=m12b_t[:, b, 4 * h : 4 * h + 4, :],
                        in_=m12_d[
                            b, 512 * h : 512 * h + 512, :
                        ].rearrange("(t p) n -> p t n", p=128),
                    )
            m12_ts = [m12b_t[:, 0], m12b_t[:, 1]]

            crow_t = constp.tile([1, 1152], f16)
            nc.sync.dma_start(out=crow_t, in_=crow_d)
            ones_t = crow_t[:, 0:128]
            bg_t = crow_t[:, 128:384]
            b1_t = crow_t[:, 384:896]
            b2_t = crow_t[:, 896:1152]
            gx_t0 = bigp.tile([128, 4, N], f16, name="gx0", tag="gxT")
            nc.sync.dma_start(
                out=gx_t0, in_=gxT_d[0].rearrange("(c p) n -> p c n", p=128)
            )
            blk_t = constp.tile([128, 1408], f16)
            nc.sync.dma_start(out=blk_t, in_=blk_d)
            ident_t = blk_t[:, 0:128]

            def wg_k(cc):
                return blk_t[:, 128 + cc * 256 : 128 + (cc + 1) * 256]

            def w1_k(cc):
                return blk_t[:, 640 + cc * 256 : 640 + (cc + 1) * 256]

            def w2_k(mc):
                return blk_t[:, 1152 + mc * 64 : 1152 + (mc + 1) * 64]

            gx_t1 = bigp.tile([128, 4, N], f16, name="gx1", tag="gxT")
            nc.sync.dma_start(
                out=gx_t1, in_=gxT_d[1].rearrange("(c p) n -> p c n", p=128)
            )
            gtT_ts = [gx_t0[:, 0:2], gx_t1[:, 0:2]]
            xT_ts = [gx_t0[:, 2:4], gx_t1[:, 2:4]]
            g1row_t = constp.tile([128, MID], bf16)
            nc.sync.dma_start(out=g1row_t, in_=g1_d.to_broadcast([128, MID]))
            if beta1_nz:
                beta1_t = constp.tile([128, MID], f32)
                nc.sync.dma_start(out=beta1_t, in_=beta1_d.to_broadcast([128, MID]))
            eps_t = constp.tile([128, 1], f32)
            nc.vector.memset(eps_t, EPS_LN)
            warm_t = constp.tile([128, 1], f16)
            nc.scalar.activation(out=warm_t, in_=eps_t, func=AF.Sigmoid)

            # ---- per-batch tile sets ----
            st = []
            for b in range(B_LOC):
                st.append({
                    "At8": bigp.tile([128, NT, N], f8, name="At8", tag="At8"),
                    "o1t": bigp.tile([128, NT, MID], f16, name="o1t", tag="o1t"),
                    "o1t8": bigp.tile([128, NT, MID], f8, name="o1t8", tag="o1t8"),
                    "o1nT": bigp.tile([128, NT, MID], f16, name="o1nT", tag="o1nT"),
                    "o1n": bigp.tile([128, 4, N], f16, name="o1n", tag="o1n"),
                    "o2t": bigp.tile([128, NT, OUT], f16, name="o2t", tag="o2t"),
                    "o2t8": bigp.tile([128, NT, OUT], f8, name="o2t8", tag="o2t8"),
                    "gts_sb": bigp.tile([128, NT, OUT], f16, name="gts_sb", tag="gts_sb"),
                    "node_sb": bigp.tile([128, NT, OUT], f16, name="node_sb", tag="node_sb"),
                })

            def stageA_jt(b, jt):
                At8 = st[b]["At8"]
                sg = workp.tile([128, N], f16, tag="sg")
                nc.scalar.activation(
                    out=sg, in_=lirow_ts[b], func=AF.Sigmoid,
                    bias=ljT_ts[b][:, jt : jt + 1], scale=1.0,
                )
                nc.vector.tensor_tensor(
                    At8[:, jt, :], m12_ts[b][:, jt, :], sg, op=OP.mult
                )

            def stageB_nt(b, nt):
                gts_sb = st[b]["gts_sb"]
                p256 = mmp.tile([128, OUT], f32, name="psB", tag="psB", bufs=2)
                nc.tensor.matmul(p256, lhsT=ones_t, rhs=bg_t, start=True, stop=False)
                for cc in range(2):
                    nc.tensor.matmul(
                        p256,
                        lhsT=gtT_ts[b][:, cc, nt * 128 : (nt + 1) * 128],
                        rhs=wg_k(cc),
                        start=False, stop=(cc == 1),
                    )
                nc.vector.tensor_scalar_max(gts_sb[:, nt, :], p256, 0.0)

            def stageC_jt(b, jt):
                o1t, o1t8 = st[b]["o1t"], st[b]["o1t8"]
                ps = mmp.tile([128, MID], f32, name="psC", tag="psC", bufs=2)
                nc.tensor.matmul(ps, lhsT=ones_t, rhs=b1_t, start=True, stop=False)
                for cc in range(2):
                    nc.tensor.matmul(
                        ps[:, cc * 256 : (cc + 1) * 256],
                        lhsT=xT_ts[b][:, cc, jt * 128 : (jt + 1) * 128],
                        rhs=w1_k(cc),
                        start=False, stop=(cc == 1),
                        skip_group_check=True,
                    )
                nc.scalar.activation(out=o1t[:, jt, :], in_=ps, func=AF.Relu)
                nc.gpsimd.dma_start(out=o1t8[:, jt, :], in_=o1t[:, jt, :])

            def stageB(b):
                for nt in range(NT):
                    stageB_nt(b, nt)
                nc.sync.dma_start(
                    out=gts_d[b].rearrange("(t p) f -> p t f", p=128),
                    in_=st[b]["gts_sb"],
                )

            def stageC(b):
                for jt in range(NT):
                    stageC_jt(b, jt)

            def stageD_mm(b):
                At8 = st[b]["At8"]
                o1t, o1t8 = st[b]["o1t"], st[b]["o1t8"]
                o1nT = st[b]["o1nT"]
                for it in range(NT):
                    ps = mmp.tile([128, MID], f32, tag="ps")
                    for jc in range(NT // 2):
                        nc.tensor.matmul(
                            ps,
                            lhsT=At8[:, 2 * jc : 2 * jc + 2, it * 128 : (it + 1) * 128],
                            rhs=o1t8[:, 2 * jc : 2 * jc + 2, :],
                            start=(jc == 0), stop=(jc == NT // 2 - 1),
                            perf_mode=DR,
                        )
                    sv = workp.tile([128, 6], f32, tag="sv")
                    nc.vector.bn_stats(out=sv, in_=ps)
                    mv = workp.tile([128, 2], f32, tag="mv")
                    nc.vector.bn_aggr(out=mv, in_=sv)
                    std = workp.tile([128, 1], f32, tag="std")
                    nc.scalar.activation(
                        out=std, in_=mv[:, 1:2], func=AF.Sqrt, bias=eps_t
                    )
                    rstd = workp.tile([128, 1], f32, tag="rstd")
                    nc.vector.reciprocal(out=rstd, in_=std)
                    outer = workp.tile([128, MID], bf16, tag="outer")
                    nc.vector.tensor_scalar_mul(outer, g1row_t, rstd)
                    nc.vector.scalar_tensor_tensor(
                        out=o1nT[:, it, :], in0=ps, scalar=mv[:, 0:1], in1=outer,
                        op0=OP.subtract, op1=OP.mult,
                    )
                    if beta1_nz:
                        nc.vector.tensor_tensor(
                            o1nT[:, it, :], o1nT[:, it, :], beta1_t, op=OP.add
                        )
                    # residual add per chunk on the DMA engines: o1nT += o1t
                    nc.gpsimd.dma_start(
                        out=o1nT[:, it, :], in_=o1t[:, it, :], accum_op=OP.add
                    )
            def stageD_tp(b):
                o1nT, o1n = st[b]["o1nT"], st[b]["o1n"]
                # transposes -> o1n [m, j]
                for it in range(NT):
                    tp = tpp.tile([128, 4, 128], f16, tag="tp")
                    for mc in range(4):
                        nc.tensor.transpose(
                            tp[:, mc, :], o1nT[:, it, mc * 128 : (mc + 1) * 128],
                            ident_t,
                        )
                    nc.scalar.activation(
                        out=o1n[:, :, it * 128 : (it + 1) * 128], in_=tp,
                        func=AF.Copy,
                    )

            def stageE(b):
                o1n, o2t, o2t8 = st[b]["o1n"], st[b]["o2t"], st[b]["o2t8"]
                for jt in range(NT):
                    ps = mmp.tile([128, MID], f32, tag="ps")
                    p256 = ps[:, :OUT]
                    nc.tensor.matmul(p256, lhsT=ones_t, rhs=b2_t, start=True, stop=False)
                    for mc in range(4):
                        nc.tensor.matmul(
                            p256[:, mc * 64 : (mc + 1) * 64],
                            lhsT=o1n[:, mc, jt * 128 : (jt + 1) * 128],
                            rhs=w2_k(mc),
                            start=False, stop=(mc == 3),
                            skip_group_check=True,
                        )
                    nc.scalar.activation(out=o2t[:, jt, :], in_=p256, func=AF.Relu)
                    nc.gpsimd.dma_start(out=o2t8[:, jt, :], in_=o2t[:, jt, :])
                nc.sync.dma_start(
                    out=o2o_d[b].rearrange("(t p) f -> p t f", p=128), in_=o2t
                )

            def stageF(b):
                At8, o2t8 = st[b]["At8"], st[b]["o2t8"]
                node_sb = st[b]["node_sb"]
                for it in range(NT):
                    ps = mmp.tile([128, MID], f32, tag="ps")
                    p256 = ps[:, :OUT]
                    for jc in range(NT // 2):
                        nc.tensor.matmul(
                            p256,
                            lhsT=At8[:, 2 * jc : 2 * jc + 2, it * 128 : (it + 1) * 128],
                            rhs=o2t8[:, 2 * jc : 2 * jc + 2, :],
                            start=(jc == 0), stop=(jc == NT // 2 - 1),
                            perf_mode=DR,
                        )
                    sv = workp.tile([128, 6], f32, tag="sv")
                    nc.vector.bn_stats(out=sv, in_=p256)
                    mv = workp.tile([128, 2], f32, tag="mv")
                    nc.vector.bn_aggr(out=mv, in_=sv)
                    std = workp.tile([128, 1], f32, tag="std")
                    nc.scalar.activation(
                        out=std, in_=mv[:, 1:2], func=AF.Sqrt, bias=eps_t
                    )
                    rstd = workp.tile([128, 1], f32, tag="rstd")
                    nc.vector.reciprocal(out=rstd, in_=std)
                    # nf = (ps - mu) * rstd  via ACT: scale=rstd, bias=-mu*rstd
                    nmu = workp.tile([128, 1], f32, tag="nmu")
                    nc.vector.tensor_tensor(nmu, mv[:, 0:1], rstd, op=OP.mult)
                    nmun = workp.tile([128, 1], f32, tag="nmun")
                    nc.vector.tensor_scalar_mul(nmun, nmu, -1.0)
                    nc.scalar.activation(
                        out=node_sb[:, it, :], in_=p256, func=AF.Identity,
                        bias=nmun, scale=rstd,
                    )
                    if it == NT // 2 - 1:
                        nc.sync.dma_start(
                            out=node_d[b, : N // 2].rearrange(
                                "(t p) f -> p t f", p=128
                            ),
                            in_=node_sb[:, : NT // 2, :],
                        )
                nc.sync.dma_start(
                    out=node_d[b, N // 2 :].rearrange("(t p) f -> p t f", p=128),
                    in_=node_sb[:, NT // 2 :, :],
                )

            # schedule: A first (sigmoid table residency); batch-1 conv work
            # fills the PE while batch-0's casts/LN drain, and vice versa.
            # phase 0: both batches' sigmoids and all of B/C interleaved.
            # sigmoid+relu share one ACT table; B-relu runs on DVE, C-relu on
            # ACT, so PSUM recycles at PE rate while the A-chain streams.
            for jt in range(NT):
                stageA_jt(0, jt)
                stageB_nt(0, jt)
                stageC_jt(0, jt)
                stageA_jt(1, jt)
                stageB_nt(1, jt)
            nc.sync.dma_start(
                out=gts_d[0].rearrange("(t p) f -> p t f", p=128),
                in_=st[0]["gts_sb"],
            )
            nc.sync.dma_start(
                out=gts_d[1].rearrange("(t p) f -> p t f", p=128),
                in_=st[1]["gts_sb"],
            )
            stageC(1)
            stageD_mm(0)
            stageD_mm(1)
            stageD_tp(0)
            stageD_tp(1)
            stageE(0)
            stageE(1)
            stageF(0)
            stageF(1)

    nc.compile()
    return nc


def _compute_col_fast(m1, m2, sm):
    """Exact col == ones proof via a cheap sufficient condition, else None."""
    if m1.min() < 0.0 or m2.min() < 0.0 or sm.min() < 0.0:
        return None
    spos = (sm > 0).astype(F32)
    colnz = np.zeros(N, dtype=bool)
    nz1max = 0.0
    nz2max = 0.0
    for b in range(B):
        p1 = (m1[b] > 0).astype(F32)
        p2 = (m2[b] > 0).astype(F32)
        nz1max = max(nz1max, float((p1 @ spos[b]).max()))
        nz2max = max(nz2max, float((p2 @ spos[b]).max()))
        colnz |= ((p1 + p2).max(axis=0) > 0) & (spos[b] > 0)
    if nz1max <= CHILDS // 4 and nz2max <= CHILDS // 2 and colnz.all():
        return np.ones(N, dtype=F32)
    return None


def _compute_col_slow(m1, m2, sm, li, lj):
    """Exact replica of the reference top-k column-union (numpy)."""
    k4, k2 = CHILDS // 4, CHILDS // 2
    col = np.zeros(N, dtype=bool)
    for b in range(B):
        logits = li[b][:, None] + lj[b][None, :]
        a = 1.0 / (1.0 + np.exp(-logits.astype(F32)))
        mr1 = m1[b] * sm[b][None, :]
        mr2 = m2[b] * sm[b][None, :]
        a1 = a * mr1
        a2 = a * mr2
        # lax.top_k ties -> lowest index; stable argsort on (-a) reproduces it.
        col[np.argsort(-a1, axis=1, kind="stable")[:, :k4].ravel()] = True
        col[np.argsort(a1, axis=1, kind="stable")[:, :k4].ravel()] = True
        col[np.argsort(-a2, axis=1, kind="stable")[:, :k2].ravel()] = True
        col[np.argsort(a2, axis=1, kind="stable")[:, :k4].ravel()] = True
    return col.astype(F32)


def kernel(**inputs):
    x = np.ascontiguousarray(np.asarray(inputs["x"], dtype=F32))
    m1 = np.asarray(inputs["masks_roi1"], dtype=F32)
    m2 = np.asarray(inputs["masks_roi2"], dtype=F32)
    sm = np.asarray(inputs["score_mask"], dtype=F32)
    gt = np.asarray(inputs["gt_feat"], dtype=F32)
    W_att = np.asarray(inputs["W_att"], dtype=F32)
    b_att = np.asarray(inputs["b_att"], dtype=F32)
    W1 = np.asarray(inputs["W1"], dtype=F32)
    b1 = np.asarray(inputs["b1"], dtype=F32)
    W2 = np.asarray(inputs["W2"], dtype=F32)
    b2 = np.asarray(inputs["b2"], dtype=F32)
    g1 = np.asarray(inputs["g1"], dtype=F32)
    beta1 = np.asarray(inputs["beta1"], dtype=F32)
    g2 = np.asarray(inputs["g2"], dtype=F32)
    beta2 = np.asarray(inputs["beta2"], dtype=F32)
    Wg = np.asarray(inputs["Wg"], dtype=F32)
    bg = np.asarray(inputs["bg"], dtype=F32)

    assert x.shape == (B, N, CIN) and W_att.shape == (2 * CIN, 1)

    # ---- host prep: tiny vector math + layout/dtype staging ----
    lj = x.reshape(B * N, CIN) @ W_att[:CIN, 0]
    lj = lj.reshape(B, N)
    li = x.reshape(B * N, CIN) @ W_att[CIN:, 0]
    li = li.reshape(B, N) + b_att[0]

    col = _compute_col_fast(m1, m2, sm)
    if col is None:
        col = _compute_col_slow(m1, m2, sm, li, lj)

    colj = sm * col[None, :]  # [B, N] multiplier along j
    m12c = (m1 + m2) * colj[:, None, :]  # [B, N(i), N(j)]
    # fold f_diag: device computes At = m12c^T(j,i) * sigmoid(z[j,i]); putting
    # fd/sigmoid(z_ii) on the diagonal yields exactly +fd there (the masked
    # diag is 0 whenever fd==1 because score_mask[i]==0 zeroes column i).
    zii = li + lj  # z[i,i] = li[i] + lj[i]
    sii = 1.0 / (1.0 + np.exp(-zii))
    fd = (sm == 0).astype(F32)
    diagval = np.minimum(fd / np.maximum(sii, 1e-6), 440.0)
    ii = np.arange(N)
    m12c[:, ii, ii] += diagval
    m12cT = np.ascontiguousarray(m12c.transpose(0, 2, 1)).astype(F8)
    gxT = np.ascontiguousarray(
        np.concatenate([gt.transpose(0, 2, 1), x.transpose(0, 2, 1)], axis=1)
    ).astype(F16)
    lirow = li.astype(F16)
    ljT = np.ascontiguousarray(lj.reshape(B, NT, 128).transpose(0, 2, 1)).astype(F32)

    # Weights: packed per-chunk transposed layouts for the grouped convs.
    w1bd = np.zeros((CIN, MID), dtype=F32)
    for g in range(G):
        w1bd[64 * g : 64 * (g + 1), 128 * g : 128 * (g + 1)] = W1[
            128 * g : 128 * (g + 1), :
        ].T
    w1K = [w1bd[128 * cc : 128 * cc + 128, 256 * cc : 256 * cc + 256] for cc in range(2)]
    w2K = [W2[64 * g : 64 * (g + 1), :].T for g in range(G)]

    blk = np.concatenate(
        [np.eye(128, dtype=F32)]
        + [Wg[128 * cc : 128 * (cc + 1), :] for cc in range(2)]
        + [w1K[cc] for cc in range(2)]
        + [w2K[mc] for mc in range(4)],
        axis=1,
    ).astype(F16)
    crow = np.concatenate(
        [np.ones(128, dtype=F32), bg, b1, b2]
    ).reshape(1, 1152).astype(F16)
    shared = {
        "constblk": np.ascontiguousarray(blk),
        "constrow": np.ascontiguousarray(crow),
        "g1row": g1.reshape(1, MID).astype(ml_dtypes.bfloat16),
        "beta1row": beta1.reshape(1, MID).astype(F32),
    }
    in_maps = []
    for c in range(NCORES):
        s = slice(B_LOC * c, B_LOC * (c + 1))
        in_maps.append(
            {
                "m12cT": m12cT[s],
                "gxT": gxT[s],
                "lirow": lirow[s],
                "ljT": ljT[s],
                **shared,
            }
        )

    beta_key = bool(np.any(beta1))
    if beta_key not in _PROGRAM_CACHE:
        _PROGRAM_CACHE[beta_key] = _build_program(beta_key)
    nc = _PROGRAM_CACHE[beta_key]

    global _LAST_IN_MAPS
    _LAST_IN_MAPS = in_maps

    from concourse.bass_utils import run_bass_kernel_spmd

    res = run_bass_kernel_spmd(nc, in_maps, core_ids=list(range(NCORES)))
    results = res.results if hasattr(res, "results") else res

    gts = np.concatenate([results[c]["gts"] for c in range(NCORES)], axis=0)
    xhat2 = np.concatenate([results[c]["node"] for c in range(NCORES)], axis=0)
    o2t = np.concatenate([results[c]["o2t"] for c in range(NCORES)], axis=0)
    gts = gts.astype(F32)
    node_feat = xhat2.astype(F32) * g2[None, None, :] + beta2[None, None, :]
    output2 = o2t.astype(F32) + node_feat
    return output2, gts, node_feat


# revision 37
# speedup vs baseline: 1.1505x; 1.1505x over previous
"""Trainium2 Bass kernel for nn_Graph_module_net_0_loss_type_18631568130084.

GNN message-passing block:
  gts       = relu(gt_feat @ Wg + bg)
  attn[i,j] = sigmoid(x[j]@Wq + x[i]@Wk + b_att)          (H == 1)
  atten     = (attn * (mr1+mr2) * col + f_diag) / CHILDS  ([B,H,Nj,Ni])
  o1 = relu(gconv1(x^T)); o1 += ln1(o1 @ atten)^T
  o2 = relu(gconv2(o1));  node_feat = ln2(o2 @ atten);  output2 = (o2 + node_feat^T)^T

Sharding: data-parallel over batch B=16 -> 2 batches per core on 8 cores.

v3 design notes (on top of the v1 transposed layout):
 * Masks pre-combined on host: m12c = (m1+m2) * (score*col)[j] in fp8e4
   ({0,1,2} and the zeros are exact).  The f_diag diagonal is folded in as
   fd[i]/sigmoid(z_ii) (exact where score==0 because the mask diag is 0
   there), so atten^T = m12c ⊙ sigmoid(z) with no device-side fixups.
 * The two big [N,N]x[N,M] contractions (stages D/F) run as fp8e4 DoubleRow
   matmuls (2 j-chunks per instruction).
 * fp16->fp8 casts of o1/o2 and the o1 residual add ride the DMA engines
   (SWDGE cast-DMA / accum-add DMA), not the compute engines.
 * ln2's gamma/beta are applied on the HOST (node_feat = xhat2*g2+b2 and
   output2 = o2 + node_feat), so stage F only standardizes.
 * Grouped convs use packed per-chunk weights (no block-diag zero columns).
 * Inputs/outputs move as one large DMA per tensor per batch.
 * The global 1/CHILDS scale cancels inside both layernorms, so it is dropped
   and eps is rescaled by CHILDS^2 to keep the math exactly equivalent.
 * The top-k "col" mask is computed exactly on the host: a cheap sufficient
   condition proves col == all-ones; otherwise an exact numpy replica runs.
"""

import numpy as np
import ml_dtypes

B = 16
N = 1024
CIN = 256
MID = 512
OUT = 256
G = 4
CHILDS = 512
NCORES = 8
B_LOC = B // NCORES  # 2
NT = N // 128  # 8
EPS_LN = 1e-6 * float(CHILDS) ** 2  # eps rescaled because we drop the 1/CHILDS

F16 = np.float16
F32 = np.float32
F8 = ml_dtypes.float8_e4m3

_PROGRAM_CACHE = {}


def _build_program(beta1_nz: bool):
    import concourse.bacc as bacc
    import concourse.bass as bass
    import concourse.tile as tile
    from concourse import mybir

    f8 = mybir.dt.float8e4
    f16 = mybir.dt.float16
    bf16 = mybir.dt.bfloat16
    f32 = mybir.dt.float32
    AF = mybir.ActivationFunctionType
    OP = mybir.AluOpType
    DR = mybir.MatmulPerfMode.DoubleRow

    nc = bacc.Bacc("TRN2", debug=False)

    def din(name, shape, dt):
        return nc.dram_tensor(name, shape, dt, kind="ExternalInput").ap()

    def dout(name, shape, dt):
        return nc.dram_tensor(name, shape, dt, kind="ExternalOutput").ap()

    # Per-core inputs (leading dim B_LOC where batch-dependent).
    m12_d = din("m12cT", [B_LOC, N, N], f8)       # (m1+m2)*(score*col) + diag, ^T
    gxT_d = din("gxT", [B_LOC, 2 * CIN, N], f16)  # [gt^T ; x^T]  [c, n]
    lirow_d = din("lirow", [B_LOC, N], f16)       # x@Wk + b_att      (per-i row)
    ljT_d = din("ljT", [B_LOC, 128, NT], f32)     # x@Wq chunked      (per-j bias)
    # Replicated weights, packed: blk = [ident | wg 2x256 | w1 2x256 | w2 4x64],
    # crow = [ones 128 | bg 256 | b1 512 | b2 256].
    blk_d = din("constblk", [128, 1408], f16)
    crow_d = din("constrow", [1, 1152], f16)
    g1_d = din("g1row", [1, MID], bf16)
    beta1_d = din("beta1row", [1, MID], f32)

    gts_d = dout("gts", [B_LOC, N, OUT], f16)
    node_d = dout("node", [B_LOC, N, OUT], f16)   # standardized xhat2 (pre-g2)
    o2o_d = dout("o2t", [B_LOC, N, OUT], f16)

    with tile.TileContext(nc) as tc:
        with tc.tile_pool(name="const", bufs=1) as constp, \
             tc.tile_pool(name="big", bufs=2) as bigp, \
             tc.tile_pool(name="work", bufs=4) as workp, \
             tc.tile_pool(name="outs", bufs=3) as outp, \
             tc.tile_pool(name="mm", bufs=3, space="PSUM") as mmp, \
             tc.tile_pool(name="tp", bufs=1, space="PSUM") as tpp:

            # ---- early DMAs: masks on the SWDGE queue, sigmoid inputs on
            # scalar, conv inputs + packed consts on sync ----
            # tiny sigmoid inputs FIRST (the big mask DMAs would otherwise
            # head-of-line-block them on the DMA engines)
            lirow_ts, ljT_ts = [], []
            for b in range(B_LOC):
                lirow_t = workp.tile([128, N], f16, tag="lirow")
                nc.scalar.dma_start(
                    out=lirow_t, in_=lirow_d[b : b + 1, :].to_broadcast([128, N])
                )
                lirow_ts.append(lirow_t)
                ljT_t = workp.tile([128, NT], f32, tag="ljT")
                nc.scalar.dma_start(out=ljT_t, in_=ljT_d[b])
                ljT_ts.append(ljT_t)
            m12b_t = bigp.tile([128, B_LOC, NT, N], f8, name="m12b", tag="m12b")
            for b in range(B_LOC):
                for h in range(2):
                    nc.gpsimd.dma_start(
                        out=m12b_t[:, b, 4 * h : 4 * h + 4, :],
                        in_=m12_d[
                            b, 512 * h : 512 * h + 512, :
                        ].rearrange("(t p) n -> p t n", p=128),
                    )
            m12_ts = [m12b_t[:, 0], m12b_t[:, 1]]

            crow_t = constp.tile([1, 1152], f16)
            nc.sync.dma_start(out=crow_t, in_=crow_d)
            ones_t = crow_t[:, 0:128]
            bg_t = crow_t[:, 128:384]
            b1_t = crow_t[:, 384:896]
            b2_t = crow_t[:, 896:1152]
            gx_t0 = bigp.tile([128, 4, N], f16, name="gx0", tag="gxT")
            nc.sync.dma_start(
                out=gx_t0, in_=gxT_d[0].rearrange("(c p) n -> p c n", p=128)
            )
            blk_t = constp.tile([128, 1408], f16)
            nc.sync.dma_start(out=blk_t, in_=blk_d)
            ident_t = blk_t[:, 0:128]

            def wg_k(cc):
                return blk_t[:, 128 + cc * 256 : 128 + (cc + 1) * 256]

            def w1_k(cc):
                return blk_t[:, 640 + cc * 256 : 640 + (cc + 1) * 256]

            def w2_k(mc):
                return blk_t[:, 1152 + mc * 64 : 1152 + (mc + 1) * 64]

            gx_t1 = bigp.tile([128, 4, N], f16, name="gx1", tag="gxT")
            nc.sync.dma_start(
                out=gx_t1, in_=gxT_d[1].rearrange("(c p) n -> p c n", p=128)
            )
            gtT_ts = [gx_t0[:, 0:2], gx_t1[:, 0:2]]
            xT_ts = [gx_t0[:, 2:4], gx_t1[:, 2:4]]
            g1row_t = constp.tile([128, MID], bf16)
            nc.sync.dma_start(out=g1row_t, in_=g1_d.to_broadcast([128, MID]))
            if beta1_nz:
                beta1_t = constp.tile([128, MID], f32)
                nc.sync.dma_start(out=beta1_t, in_=beta1_d.to_broadcast([128, MID]))
            eps_t = constp.tile([128, 1], f32)
            nc.vector.memset(eps_t, EPS_LN)
            warm_t = constp.tile([128, 1], f16)
            nc.scalar.activation(out=warm_t, in_=eps_t, func=AF.Sigmoid)

            # ---- per-batch tile sets ----
            st = []
            for b in range(B_LOC):
                st.append({
                    "At8": bigp.tile([128, NT, N], f8, name="At8", tag="At8"),
                    "o1t": bigp.tile([128, NT, MID], f16, name="o1t", tag="o1t"),
                    "o1t8": bigp.tile([128, NT, MID], f8, name="o1t8", tag="o1t8"),
                    "o1nT": bigp.tile([128, NT, MID], f16, name="o1nT", tag="o1nT"),
                    "o1n": bigp.tile([128, 4, N], f16, name="o1n", tag="o1n"),
                    "o2t": bigp.tile([128, NT, OUT], f16, name="o2t", tag="o2t"),
                    "o2t8": bigp.tile([128, NT, OUT], f8, name="o2t8", tag="o2t8"),
                    "gts_sb": bigp.tile([128, NT, OUT], f16, name="gts_sb", tag="gts_sb"),
                    "node_sb": bigp.tile([128, NT, OUT], f16, name="node_sb", tag="node_sb"),
                })

            def stageA_jt(b, jt):
                At8 = st[b]["At8"]
                sg = workp.tile([128, N], f16, tag="sg")
                nc.scalar.activation(
                    out=sg, in_=lirow_ts[b], func=AF.Sigmoid,
                    bias=ljT_ts[b][:, jt : jt + 1], scale=1.0,
                )
                nc.vector.tensor_tensor(
                    At8[:, jt, :], m12_ts[b][:, jt, :], sg, op=OP.mult
                )

            def stageB_nt(b, nt):
                gts_sb = st[b]["gts_sb"]
                p256 = mmp.tile([128, OUT], f32, name="psB", tag="psB", bufs=2)
                nc.tensor.matmul(p256, lhsT=ones_t, rhs=bg_t, start=True, stop=False)
                for cc in range(2):
                    nc.tensor.matmul(
                        p256,
                        lhsT=gtT_ts[b][:, cc, nt * 128 : (nt + 1) * 128],
                        rhs=wg_k(cc),
                        start=False, stop=(cc == 1),
                    )
                nc.vector.tensor_scalar_max(gts_sb[:, nt, :], p256, 0.0)

            def stageC_jt(b, jt):
                o1t, o1t8 = st[b]["o1t"], st[b]["o1t8"]
                ps = mmp.tile([128, MID], f32, name="psC", tag="psC", bufs=2)
                nc.tensor.matmul(ps, lhsT=ones_t, rhs=b1_t, start=True, stop=False)
                for cc in range(2):
                    nc.tensor.matmul(
                        ps[:, cc * 256 : (cc + 1) * 256],
                        lhsT=xT_ts[b][:, cc, jt * 128 : (jt + 1) * 128],
                        rhs=w1_k(cc),
                        start=False, stop=(cc == 1),
                        skip_group_check=True,
                    )
                nc.scalar.activation(out=o1t[:, jt, :], in_=ps, func=AF.Relu)
                nc.gpsimd.dma_start(out=o1t8[:, jt, :], in_=o1t[:, jt, :])

            def stageB(b):
                for nt in range(NT):
                    stageB_nt(b, nt)
                nc.sync.dma_start(
                    out=gts_d[b].rearrange("(t p) f -> p t f", p=128),
                    in_=st[b]["gts_sb"],
                )

            def stageC(b):
                for jt in range(NT):
                    stageC_jt(b, jt)

            def stageD_mm(b):
                At8 = st[b]["At8"]
                o1t, o1t8 = st[b]["o1t"], st[b]["o1t8"]
                o1nT = st[b]["o1nT"]
                for it in range(NT):
                    ps = mmp.tile([128, MID], f32, tag="ps")
                    for jc in range(NT // 2):
                        nc.tensor.matmul(
                            ps,
                            lhsT=At8[:, 2 * jc : 2 * jc + 2, it * 128 : (it + 1) * 128],
                            rhs=o1t8[:, 2 * jc : 2 * jc + 2, :],
                            start=(jc == 0), stop=(jc == NT // 2 - 1),
                            perf_mode=DR,
                        )
                    sv = workp.tile([128, 6], f32, tag="sv")
                    nc.vector.bn_stats(out=sv, in_=ps)
                    mv = workp.tile([128, 2], f32, tag="mv")
                    nc.vector.bn_aggr(out=mv, in_=sv)
                    std = workp.tile([128, 1], f32, tag="std")
                    nc.scalar.activation(
                        out=std, in_=mv[:, 1:2], func=AF.Sqrt, bias=eps_t
                    )
                    rstd = workp.tile([128, 1], f32, tag="rstd")
                    nc.vector.reciprocal(out=rstd, in_=std)
                    outer = workp.tile([128, MID], bf16, tag="outer")
                    nc.vector.tensor_scalar_mul(outer, g1row_t, rstd)
                    nc.vector.scalar_tensor_tensor(
                        out=o1nT[:, it, :], in0=ps, scalar=mv[:, 0:1], in1=outer,
                        op0=OP.subtract, op1=OP.mult,
                    )
                    if beta1_nz:
                        nc.vector.tensor_tensor(
                            o1nT[:, it, :], o1nT[:, it, :], beta1_t, op=OP.add
                        )
                    # residual add per chunk on the DMA engines: o1nT += o1t
                    nc.gpsimd.dma_start(
                        out=o1nT[:, it, :], in_=o1t[:, it, :], accum_op=OP.add
                    )
            def stageD_tp(b):
                o1nT, o1n = st[b]["o1nT"], st[b]["o1n"]
                # transposes -> o1n [m, j]
                for it in range(NT):
                    tp = tpp.tile([128, 4, 128], f16, tag="tp")
                    for mc in range(4):
                        nc.tensor.transpose(
                            tp[:, mc, :], o1nT[:, it, mc * 128 : (mc + 1) * 128],
                            ident_t,
                        )
                    nc.scalar.activation(
                        out=o1n[:, :, it * 128 : (it + 1) * 128], in_=tp,
                        func=AF.Copy,
                    )

            def stageE(b):
                o1n, o2t, o2t8 = st[b]["o1n"], st[b]["o2t"], st[b]["o2t8"]
                for jt in range(NT):
                    ps = mmp.tile([128, MID], f32, tag="ps")
                    p256 = ps[:, :OUT]
                    nc.tensor.matmul(p256, lhsT=ones_t, rhs=b2_t, start=True, stop=False)
                    for mc in range(4):
                        nc.tensor.matmul(
                            p256[:, mc * 64 : (mc + 1) * 64],
                            lhsT=o1n[:, mc, jt * 128 : (jt + 1) * 128],
                            rhs=w2_k(mc),
                            start=False, stop=(mc == 3),
                            skip_group_check=True,
                        )
                    nc.scalar.activation(out=o2t[:, jt, :], in_=p256, func=AF.Relu)
                    nc.gpsimd.dma_start(out=o2t8[:, jt, :], in_=o2t[:, jt, :])
                nc.sync.dma_start(
                    out=o2o_d[b].rearrange("(t p) f -> p t f", p=128), in_=o2t
                )

            def stageF(b):
                At8, o2t8 = st[b]["At8"], st[b]["o2t8"]
                node_sb = st[b]["node_sb"]
                for it in range(NT):
                    ps = mmp.tile([128, MID], f32, tag="ps")
                    p256 = ps[:, :OUT]
                    for jc in range(NT // 2):
                        nc.tensor.matmul(
                            p256,
                            lhsT=At8[:, 2 * jc : 2 * jc + 2, it * 128 : (it + 1) * 128],
                            rhs=o2t8[:, 2 * jc : 2 * jc + 2, :],
                            start=(jc == 0), stop=(jc == NT // 2 - 1),
                            perf_mode=DR,
                        )
                    sv = workp.tile([128, 6], f32, tag="sv")
                    nc.vector.bn_stats(out=sv, in_=p256)
                    mv = workp.tile([128, 2], f32, tag="mv")
                    nc.vector.bn_aggr(out=mv, in_=sv)
                    std = workp.tile([128, 1], f32, tag="std")
                    nc.scalar.activation(
                        out=std, in_=mv[:, 1:2], func=AF.Sqrt, bias=eps_t
                    )
                    rstd = workp.tile([128, 1], f32, tag="rstd")
                    nc.vector.reciprocal(out=rstd, in_=std)
                    # nf = (ps - mu) * rstd  via ACT: scale=rstd, bias=-mu*rstd
                    nmu = workp.tile([128, 1], f32, tag="nmu")
                    nc.vector.tensor_tensor(nmu, mv[:, 0:1], rstd, op=OP.mult)
                    nmun = workp.tile([128, 1], f32, tag="nmun")
                    nc.vector.tensor_scalar_mul(nmun, nmu, -1.0)
                    nc.scalar.activation(
                        out=node_sb[:, it, :], in_=p256, func=AF.Identity,
                        bias=nmun, scale=rstd,
                    )
                    if it == NT // 2 - 1:
                        nc.sync.dma_start(
                            out=node_d[b, : N // 2].rearrange(
                                "(t p) f -> p t f", p=128
                            ),
                            in_=node_sb[:, : NT // 2, :],
                        )
                nc.sync.dma_start(
                    out=node_d[b, N // 2 :].rearrange("(t p) f -> p t f", p=128),
                    in_=node_sb[:, NT // 2 :, :],
                )

            # schedule: A first (sigmoid table residency); batch-1 conv work
            # fills the PE while batch-0's casts/LN drain, and vice versa.
            # phase 0: both batches' sigmoids and all of B/C interleaved.
            # sigmoid+relu share one ACT table; B-relu runs on DVE, C-relu on
            # ACT, so PSUM recycles at PE rate while the A-chain streams.
            for jt in range(NT):
                stageA_jt(0, jt)
                stageB_nt(0, jt)
                stageC_jt(0, jt)
                stageA_jt(1, jt)
                stageB_nt(1, jt)
            nc.sync.dma_start(
                out=gts_d[0].rearrange("(t p) f -> p t f", p=128),
                in_=st[0]["gts_sb"],
            )
            nc.sync.dma_start(
                out=gts_d[1].rearrange("(t p) f -> p t f", p=128),
                in_=st[1]["gts_sb"],
            )
            stageC(1)
            stageD_mm(0)
            stageD_mm(1)
            stageD_tp(0)
            stageD_tp(1)
            stageE(0)
            stageE(1)
            stageF(0)
            stageF(1)

    nc.compile()
    return nc


def _compute_col_fast(m1, m2, sm):
    """Exact col == ones proof via a cheap sufficient condition, else None."""
    if m1.min() < 0.0 or m2.min() < 0.0 or sm.min() < 0.0:
        return None
    spos = (sm > 0).astype(F32)
    colnz = np.zeros(N, dtype=bool)
    nz1max = 0.0
    nz2max = 0.0
    for b in range(B):
        p1 = (m1[b] > 0).astype(F32)
        p2 = (m2[b] > 0).astype(F32)
        nz1max = max(nz1max, float((p1 @ spos[b]).max()))
        nz2max = max(nz2max, float((p2 @ spos[b]).max()))
        colnz |= ((p1 + p2).max(axis=0) > 0) & (spos[b] > 0)
    if nz1max <= CHILDS // 4 and nz2max <= CHILDS // 2 and colnz.all():
        return np.ones(N, dtype=F32)
    return None


def _compute_col_slow(m1, m2, sm, li, lj):
    """Exact replica of the reference top-k column-union (numpy)."""
    k4, k2 = CHILDS // 4, CHILDS // 2
    col = np.zeros(N, dtype=bool)
    for b in range(B):
        logits = li[b][:, None] + lj[b][None, :]
        a = 1.0 / (1.0 + np.exp(-logits.astype(F32)))
        mr1 = m1[b] * sm[b][None, :]
        mr2 = m2[b] * sm[b][None, :]
        a1 = a * mr1
        a2 = a * mr2
        # lax.top_k ties -> lowest index; stable argsort on (-a) reproduces it.
        col[np.argsort(-a1, axis=1, kind="stable")[:, :k4].ravel()] = True
        col[np.argsort(a1, axis=1, kind="stable")[:, :k4].ravel()] = True
        col[np.argsort(-a2, axis=1, kind="stable")[:, :k2].ravel()] = True
        col[np.argsort(a2, axis=1, kind="stable")[:, :k4].ravel()] = True
    return col.astype(F32)


def kernel(**inputs):
    x = np.ascontiguousarray(np.asarray(inputs["x"], dtype=F32))
    m1 = np.asarray(inputs["masks_roi1"], dtype=F32)
    m2 = np.asarray(inputs["masks_roi2"], dtype=F32)
    sm = np.asarray(inputs["score_mask"], dtype=F32)
    gt = np.asarray(inputs["gt_feat"], dtype=F32)
    W_att = np.asarray(inputs["W_att"], dtype=F32)
    b_att = np.asarray(inputs["b_att"], dtype=F32)
    W1 = np.asarray(inputs["W1"], dtype=F32)
    b1 = np.asarray(inputs["b1"], dtype=F32)
    W2 = np.asarray(inputs["W2"], dtype=F32)
    b2 = np.asarray(inputs["b2"], dtype=F32)
    g1 = np.asarray(inputs["g1"], dtype=F32)
    beta1 = np.asarray(inputs["beta1"], dtype=F32)
    g2 = np.asarray(inputs["g2"], dtype=F32)
    beta2 = np.asarray(inputs["beta2"], dtype=F32)
    Wg = np.asarray(inputs["Wg"], dtype=F32)
    bg = np.asarray(inputs["bg"], dtype=F32)

    assert x.shape == (B, N, CIN) and W_att.shape == (2 * CIN, 1)

    # ---- host prep: tiny vector math + layout/dtype staging ----
    lj = x.reshape(B * N, CIN) @ W_att[:CIN, 0]
    lj = lj.reshape(B, N)
    li = x.reshape(B * N, CIN) @ W_att[CIN:, 0]
    li = li.reshape(B, N) + b_att[0]

    col = _compute_col_fast(m1, m2, sm)
    if col is None:
        col = _compute_col_slow(m1, m2, sm, li, lj)

    colj = sm * col[None, :]  # [B, N] multiplier along j
    m12c = (m1 + m2) * colj[:, None, :]  # [B, N(i), N(j)]
    # fold f_diag: device computes At = m12c^T(j,i) * sigmoid(z[j,i]); putting
    # fd/sigmoid(z_ii) on the diagonal yields exactly +fd there (the masked
    # diag is 0 whenever fd==1 because score_mask[i]==0 zeroes column i).
    zii = li + lj  # z[i,i] = li[i] + lj[i]
    sii = 1.0 / (1.0 + np.exp(-zii))
    fd = (sm == 0).astype(F32)
    diagval = np.minimum(fd / np.maximum(sii, 1e-6), 440.0)
    ii = np.arange(N)
    m12c[:, ii, ii] += diagval
    m12cT = np.ascontiguousarray(m12c.transpose(0, 2, 1)).astype(F8)
    gxT = np.ascontiguousarray(
        np.concatenate([gt.transpose(0, 2, 1), x.transpose(0, 2, 1)], axis=1)
    ).astype(F16)
    lirow = li.astype(F16)
    ljT = np.ascontiguousarray(lj.reshape(B, NT, 128).transpose(0, 2, 1)).astype(F32)

    # Weights: packed per-chunk transposed layouts for the grouped convs.
    w1bd = np.zeros((CIN, MID), dtype=F32)
    for g in range(G):
        w1bd[64 * g : 64 * (g + 1), 128 * g : 128 * (g + 1)] = W1[
            128 * g : 128 * (g + 1), :
        ].T
    w1K = [w1bd[128 * cc : 128 * cc + 128, 256 * cc : 256 * cc + 256] for cc in range(2)]
    w2K = [W2[64 * g : 64 * (g + 1), :].T for g in range(G)]

    blk = np.concatenate(
        [np.eye(128, dtype=F32)]
        + [Wg[128 * cc : 128 * (cc + 1), :] for cc in range(2)]
        + [w1K[cc] for cc in range(2)]
        + [w2K[mc] for mc in range(4)],
        axis=1,
    ).astype(F16)
    crow = np.concatenate(
        [np.ones(128, dtype=F32), bg, b1, b2]
    ).reshape(1, 1152).astype(F16)
    shared = {
        "constblk": np.ascontiguousarray(blk),
        "constrow": np.ascontiguousarray(crow),
        "g1row": g1.reshape(1, MID).astype(ml_dtypes.bfloat16),
        "beta1row": beta1.reshape(1, MID).astype(F32),
    }
    in_maps = []
    for c in range(NCORES):
        s = slice(B_LOC * c, B_LOC * (c + 1))
        in_maps.append(
            {
                "m12cT": m12cT[s],
                "gxT": gxT[s],
                "lirow": lirow[s],
                "ljT": ljT[s],
                **shared,
            }
        )

    beta_key = bool(np.any(beta1))
    if beta_key not in _PROGRAM_CACHE:
        _PROGRAM_CACHE[beta_key] = _build_program(beta_key)
    nc = _PROGRAM_CACHE[beta_key]

    global _LAST_IN_MAPS
    _LAST_IN_MAPS = in_maps

    from concourse.bass_utils import run_bass_kernel_spmd

    res = run_bass_kernel_spmd(nc, in_maps, core_ids=list(range(NCORES)))
    results = res.results if hasattr(res, "results") else res

    gts = np.concatenate([results[c]["gts"] for c in range(NCORES)], axis=0)
    xhat2 = np.concatenate([results[c]["node"] for c in range(NCORES)], axis=0)
    o2t = np.concatenate([results[c]["o2t"] for c in range(NCORES)], axis=0)
    gts = gts.astype(F32)
    node_feat = xhat2.astype(F32) * g2[None, None, :] + beta2[None, None, :]
    output2 = o2t.astype(F32) + node_feat
    return output2, gts, node_feat


# revision 38
# speedup vs baseline: 1.2487x; 1.0854x over previous
"""Trainium2 Bass kernel for nn_Graph_module_net_0_loss_type_18631568130084.

GNN message-passing block:
  gts       = relu(gt_feat @ Wg + bg)
  attn[i,j] = sigmoid(x[j]@Wq + x[i]@Wk + b_att)          (H == 1)
  atten     = (attn * (mr1+mr2) * col + f_diag) / CHILDS  ([B,H,Nj,Ni])
  o1 = relu(gconv1(x^T)); o1 += ln1(o1 @ atten)^T
  o2 = relu(gconv2(o1));  node_feat = ln2(o2 @ atten);  output2 = (o2 + node_feat^T)^T

Sharding: data-parallel over batch B=16 -> 2 batches per core on 8 cores.

v3 design notes (on top of the v1 transposed layout):
 * Masks pre-combined on host: m12c = (m1+m2) * (score*col)[j] in fp8e4
   ({0,1,2} and the zeros are exact).  The f_diag diagonal is folded in as
   fd[i]/sigmoid(z_ii) (exact where score==0 because the mask diag is 0
   there), so atten^T = m12c ⊙ sigmoid(z) with no device-side fixups.
 * The two big [N,N]x[N,M] contractions (stages D/F) run as fp8e4 DoubleRow
   matmuls (2 j-chunks per instruction).
 * fp16->fp8 casts of o1/o2 and the o1 residual add ride the DMA engines
   (SWDGE cast-DMA / accum-add DMA), not the compute engines.
 * ln2's gamma/beta are applied on the HOST (node_feat = xhat2*g2+b2 and
   output2 = o2 + node_feat), so stage F only standardizes.
 * Grouped convs use packed per-chunk weights (no block-diag zero columns).
 * Inputs/outputs move as one large DMA per tensor per batch.
 * The global 1/CHILDS scale cancels inside both layernorms, so it is dropped
   and eps is rescaled by CHILDS^2 to keep the math exactly equivalent.
 * The top-k "col" mask is computed exactly on the host: a cheap sufficient
   condition proves col == all-ones; otherwise an exact numpy replica runs.
"""

import numpy as np
import ml_dtypes

B = 16
N = 1024
CIN = 256
MID = 512
OUT = 256
G = 4
CHILDS = 512
NCORES = 8
B_LOC = B // NCORES  # 2
NT = N // 128  # 8
EPS_LN = 1e-6 * float(CHILDS) ** 2  # eps rescaled because we drop the 1/CHILDS

F16 = np.float16
F32 = np.float32
F8 = ml_dtypes.float8_e4m3

_PROGRAM_CACHE = {}


def _build_program(beta1_nz: bool):
    import concourse.bacc as bacc
    import concourse.bass as bass
    import concourse.tile as tile
    from concourse import mybir

    f8 = mybir.dt.float8e4
    f16 = mybir.dt.float16
    bf16 = mybir.dt.bfloat16
    f32 = mybir.dt.float32
    AF = mybir.ActivationFunctionType
    OP = mybir.AluOpType
    DR = mybir.MatmulPerfMode.DoubleRow

    nc = bacc.Bacc("TRN2", debug=False)

    def din(name, shape, dt):
        return nc.dram_tensor(name, shape, dt, kind="ExternalInput").ap()

    def dout(name, shape, dt):
        return nc.dram_tensor(name, shape, dt, kind="ExternalOutput").ap()

    # Per-core inputs (leading dim B_LOC where batch-dependent).
    m12_d = din("m12cT", [B_LOC, N, N], f8)       # (m1+m2)*(score*col) + diag, ^T
    gxT_d = din("gxT", [B_LOC, 2 * CIN, N], f16)  # [gt^T ; x^T]  [c, n]
    lirow_d = din("lirow", [B_LOC, N], f16)       # x@Wk + b_att      (per-i row)
    ljT_d = din("ljT", [B_LOC, 128, NT], f32)     # x@Wq chunked      (per-j bias)
    # Replicated weights, packed: blk = [ident | wg 2x256 | w1 2x256 | w2 4x64],
    # crow = [ones 128 | bg 256 | b1 512 | b2 256].
    blk_d = din("constblk", [128, 1408], f16)
    crow_d = din("constrow", [1, 1152], f16)
    g1_d = din("g1row", [1, MID], bf16)
    beta1_d = din("beta1row", [1, MID], f32)

    gts_d = dout("gts", [B_LOC, N, OUT], f16)
    node_d = dout("node", [B_LOC, N, OUT], f16)   # standardized xhat2 (pre-g2)
    o2o_d = dout("o2t", [B_LOC, N, OUT], f16)

    with tile.TileContext(nc) as tc:
        with tc.tile_pool(name="const", bufs=1) as constp, \
             tc.tile_pool(name="big", bufs=2) as bigp, \
             tc.tile_pool(name="work", bufs=4) as workp, \
             tc.tile_pool(name="outs", bufs=3) as outp, \
             tc.tile_pool(name="mm", bufs=5, space="PSUM") as mmp, \
             tc.tile_pool(name="tp", bufs=2, space="PSUM") as tpp:

            # ---- early DMAs: masks on the SWDGE queue, sigmoid inputs on
            # scalar, conv inputs + packed consts on sync ----
            # tiny sigmoid inputs FIRST (the big mask DMAs would otherwise
            # head-of-line-block them on the DMA engines)
            lirow_ts, ljT_ts = [], []
            for b in range(B_LOC):
                lirow_t = workp.tile([128, N], f16, tag="lirow")
                nc.scalar.dma_start(
                    out=lirow_t, in_=lirow_d[b : b + 1, :].to_broadcast([128, N])
                )
                lirow_ts.append(lirow_t)
                ljT_t = workp.tile([128, NT], f32, tag="ljT")
                nc.scalar.dma_start(out=ljT_t, in_=ljT_d[b])
                ljT_ts.append(ljT_t)
            m12b_t = bigp.tile([128, B_LOC, NT, N], f8, name="m12b", tag="m12b")
            for b in range(B_LOC):
                for h in range(2):
                    nc.gpsimd.dma_start(
                        out=m12b_t[:, b, 4 * h : 4 * h + 4, :],
                        in_=m12_d[
                            b, 512 * h : 512 * h + 512, :
                        ].rearrange("(t p) n -> p t n", p=128),
                    )
            m12_ts = [m12b_t[:, 0], m12b_t[:, 1]]

            crow_t = constp.tile([1, 1152], f16)
            nc.sync.dma_start(out=crow_t, in_=crow_d)
            ones_t = crow_t[:, 0:128]
            bg_t = crow_t[:, 128:384]
            b1_t = crow_t[:, 384:896]
            b2_t = crow_t[:, 896:1152]
            gx_t0 = bigp.tile([128, 4, N], f16, name="gx0", tag="gxT")
            nc.sync.dma_start(
                out=gx_t0, in_=gxT_d[0].rearrange("(c p) n -> p c n", p=128)
            )
            blk_t = constp.tile([128, 1408], f16)
            nc.sync.dma_start(out=blk_t, in_=blk_d)
            ident_t = blk_t[:, 0:128]

            def wg_k(cc):
                return blk_t[:, 128 + cc * 256 : 128 + (cc + 1) * 256]

            def w1_k(cc):
                return blk_t[:, 640 + cc * 256 : 640 + (cc + 1) * 256]

            def w2_k(mc):
                return blk_t[:, 1152 + mc * 64 : 1152 + (mc + 1) * 64]

            gx_t1 = bigp.tile([128, 4, N], f16, name="gx1", tag="gxT")
            nc.sync.dma_start(
                out=gx_t1, in_=gxT_d[1].rearrange("(c p) n -> p c n", p=128)
            )
            gtT_ts = [gx_t0[:, 0:2], gx_t1[:, 0:2]]
            xT_ts = [gx_t0[:, 2:4], gx_t1[:, 2:4]]
            g1row_t = constp.tile([128, MID], bf16)
            nc.sync.dma_start(out=g1row_t, in_=g1_d.to_broadcast([128, MID]))
            if beta1_nz:
                beta1_t = constp.tile([128, MID], f32)
                nc.sync.dma_start(out=beta1_t, in_=beta1_d.to_broadcast([128, MID]))
            eps_t = constp.tile([128, 1], f32)
            nc.vector.memset(eps_t, EPS_LN)
            warm_t = constp.tile([128, 1], f16)
            nc.scalar.activation(out=warm_t, in_=eps_t, func=AF.Sigmoid)

            # ---- per-batch tile sets ----
            st = []
            for b in range(B_LOC):
                st.append({
                    "At8": bigp.tile([128, NT, N], f8, name="At8", tag="At8"),
                    "o1t": bigp.tile([128, NT, MID], f16, name="o1t", tag="o1t"),
                    "o1t8": bigp.tile([128, NT, MID], f8, name="o1t8", tag="o1t8"),
                    "o1nT": bigp.tile([128, NT, MID], f16, name="o1nT", tag="o1nT"),
                    "o1n": bigp.tile([128, 4, N], f16, name="o1n", tag="o1n"),
                    "o2t": bigp.tile([128, NT, OUT], f16, name="o2t", tag="o2t"),
                    "o2t8": bigp.tile([128, NT, OUT], f8, name="o2t8", tag="o2t8"),
                    "gts_sb": bigp.tile([128, NT, OUT], f16, name="gts_sb", tag="gts_sb"),
                    "node_sb": bigp.tile([128, NT, OUT], f16, name="node_sb", tag="node_sb"),
                })

            def stageA_jt(b, jt):
                At8 = st[b]["At8"]
                sg = workp.tile([128, N], f16, tag="sg")
                nc.scalar.activation(
                    out=sg, in_=lirow_ts[b], func=AF.Sigmoid,
                    bias=ljT_ts[b][:, jt : jt + 1], scale=1.0,
                )
                nc.vector.tensor_tensor(
                    At8[:, jt, :], m12_ts[b][:, jt, :], sg, op=OP.mult
                )

            def stageB_nt(b, nt):
                gts_sb = st[b]["gts_sb"]
                ps = mmp.tile([128, MID], f32, tag="ps")
                p256 = ps[:, :OUT]
                nc.tensor.matmul(p256, lhsT=ones_t, rhs=bg_t, start=True, stop=False)
                for cc in range(2):
                    nc.tensor.matmul(
                        p256,
                        lhsT=gtT_ts[b][:, cc, nt * 128 : (nt + 1) * 128],
                        rhs=wg_k(cc),
                        start=False, stop=(cc == 1),
                    )
                nc.vector.tensor_scalar_max(gts_sb[:, nt, :], p256, 0.0)

            def stageC_jt(b, jt):
                o1t, o1t8 = st[b]["o1t"], st[b]["o1t8"]
                ps = mmp.tile([128, MID], f32, tag="ps")
                nc.tensor.matmul(ps, lhsT=ones_t, rhs=b1_t, start=True, stop=False)
                for cc in range(2):
                    nc.tensor.matmul(
                        ps[:, cc * 256 : (cc + 1) * 256],
                        lhsT=xT_ts[b][:, cc, jt * 128 : (jt + 1) * 128],
                        rhs=w1_k(cc),
                        start=False, stop=(cc == 1),
                        skip_group_check=True,
                    )
                nc.scalar.activation(out=o1t[:, jt, :], in_=ps, func=AF.Relu)
                nc.gpsimd.dma_start(out=o1t8[:, jt, :], in_=o1t[:, jt, :])

            def stageB(b):
                for nt in range(NT):
                    stageB_nt(b, nt)
                nc.sync.dma_start(
                    out=gts_d[b].rearrange("(t p) f -> p t f", p=128),
                    in_=st[b]["gts_sb"],
                )

            def stageC(b):
                for jt in range(NT):
                    stageC_jt(b, jt)

            def stageD_mm(b):
                At8 = st[b]["At8"]
                o1t, o1t8 = st[b]["o1t"], st[b]["o1t8"]
                o1nT = st[b]["o1nT"]
                for it in range(NT):
                    ps = mmp.tile([128, MID], f32, tag="ps")
                    for jc in range(NT // 2):
                        nc.tensor.matmul(
                            ps,
                            lhsT=At8[:, 2 * jc : 2 * jc + 2, it * 128 : (it + 1) * 128],
                            rhs=o1t8[:, 2 * jc : 2 * jc + 2, :],
                            start=(jc == 0), stop=(jc == NT // 2 - 1),
                            perf_mode=DR,
                        )
                    sv = workp.tile([128, 6], f32, tag="sv")
                    nc.vector.bn_stats(out=sv, in_=ps)
                    mv = workp.tile([128, 2], f32, tag="mv")
                    nc.vector.bn_aggr(out=mv, in_=sv)
                    std = workp.tile([128, 1], f32, tag="std")
                    nc.scalar.activation(
                        out=std, in_=mv[:, 1:2], func=AF.Sqrt, bias=eps_t
                    )
                    rstd = workp.tile([128, 1], f32, tag="rstd")
                    nc.vector.reciprocal(out=rstd, in_=std)
                    outer = workp.tile([128, MID], bf16, tag="outer")
                    nc.vector.tensor_scalar_mul(outer, g1row_t, rstd)
                    nc.vector.scalar_tensor_tensor(
                        out=o1nT[:, it, :], in0=ps, scalar=mv[:, 0:1], in1=outer,
                        op0=OP.subtract, op1=OP.mult,
                    )
                    if beta1_nz:
                        nc.vector.tensor_tensor(
                            o1nT[:, it, :], o1nT[:, it, :], beta1_t, op=OP.add
                        )
                    # residual add per chunk on the DMA engines: o1nT += o1t
                    nc.gpsimd.dma_start(
                        out=o1nT[:, it, :], in_=o1t[:, it, :], accum_op=OP.add
                    )
            def stageD_tp(b):
                o1nT, o1n = st[b]["o1nT"], st[b]["o1n"]
                # transposes -> o1n [m, j]
                for it in range(NT):
                    tp = tpp.tile([128, 4, 128], f16, tag="tp")
                    for mc in range(4):
                        nc.tensor.transpose(
                            tp[:, mc, :], o1nT[:, it, mc * 128 : (mc + 1) * 128],
                            ident_t,
                        )
                    nc.scalar.activation(
                        out=o1n[:, :, it * 128 : (it + 1) * 128], in_=tp,
                        func=AF.Copy,
                    )

            def stageE(b):
                o1n, o2t, o2t8 = st[b]["o1n"], st[b]["o2t"], st[b]["o2t8"]
                for jt in range(NT):
                    ps = mmp.tile([128, MID], f32, tag="ps")
                    p256 = ps[:, :OUT]
                    nc.tensor.matmul(p256, lhsT=ones_t, rhs=b2_t, start=True, stop=False)
                    for mc in range(4):
                        nc.tensor.matmul(
                            p256[:, mc * 64 : (mc + 1) * 64],
                            lhsT=o1n[:, mc, jt * 128 : (jt + 1) * 128],
                            rhs=w2_k(mc),
                            start=False, stop=(mc == 3),
                            skip_group_check=True,
                        )
                    nc.scalar.activation(out=o2t[:, jt, :], in_=p256, func=AF.Relu)
                    nc.gpsimd.dma_start(out=o2t8[:, jt, :], in_=o2t[:, jt, :])
                nc.sync.dma_start(
                    out=o2o_d[b].rearrange("(t p) f -> p t f", p=128), in_=o2t
                )

            def stageF(b):
                At8, o2t8 = st[b]["At8"], st[b]["o2t8"]
                node_sb = st[b]["node_sb"]
                for it in range(NT):
                    ps = mmp.tile([128, MID], f32, tag="ps")
                    p256 = ps[:, :OUT]
                    for jc in range(NT // 2):
                        nc.tensor.matmul(
                            p256,
                            lhsT=At8[:, 2 * jc : 2 * jc + 2, it * 128 : (it + 1) * 128],
                            rhs=o2t8[:, 2 * jc : 2 * jc + 2, :],
                            start=(jc == 0), stop=(jc == NT // 2 - 1),
                            perf_mode=DR,
                        )
                    sv = workp.tile([128, 6], f32, tag="sv")
                    nc.vector.bn_stats(out=sv, in_=p256)
                    mv = workp.tile([128, 2], f32, tag="mv")
                    nc.vector.bn_aggr(out=mv, in_=sv)
                    std = workp.tile([128, 1], f32, tag="std")
                    nc.scalar.activation(
                        out=std, in_=mv[:, 1:2], func=AF.Sqrt, bias=eps_t
                    )
                    rstd = workp.tile([128, 1], f32, tag="rstd")
                    nc.vector.reciprocal(out=rstd, in_=std)
                    # nf = (ps - mu) * rstd  via ACT: scale=rstd, bias=-mu*rstd
                    nmu = workp.tile([128, 1], f32, tag="nmu")
                    nc.vector.tensor_tensor(nmu, mv[:, 0:1], rstd, op=OP.mult)
                    nmun = workp.tile([128, 1], f32, tag="nmun")
                    nc.vector.tensor_scalar_mul(nmun, nmu, -1.0)
                    nc.scalar.activation(
                        out=node_sb[:, it, :], in_=p256, func=AF.Identity,
                        bias=nmun, scale=rstd,
                    )
                    if it == NT // 2 - 1:
                        nc.sync.dma_start(
                            out=node_d[b, : N // 2].rearrange(
                                "(t p) f -> p t f", p=128
                            ),
                            in_=node_sb[:, : NT // 2, :],
                        )
                nc.sync.dma_start(
                    out=node_d[b, N // 2 :].rearrange("(t p) f -> p t f", p=128),
                    in_=node_sb[:, NT // 2 :, :],
                )

            # schedule: A first (sigmoid table residency); batch-1 conv work
            # fills the PE while batch-0's casts/LN drain, and vice versa.
            # phase 0: both batches' sigmoids and all of B/C interleaved.
            # sigmoid+relu share one ACT table; B-relu runs on DVE, C-relu on
            # ACT, so PSUM recycles at PE rate while the A-chain streams.
            for jt in range(NT):
                stageA_jt(0, jt)
                stageB_nt(0, jt)
                stageC_jt(0, jt)
                stageA_jt(1, jt)
                stageB_nt(1, jt)
            nc.sync.dma_start(
                out=gts_d[0].rearrange("(t p) f -> p t f", p=128),
                in_=st[0]["gts_sb"],
            )
            nc.sync.dma_start(
                out=gts_d[1].rearrange("(t p) f -> p t f", p=128),
                in_=st[1]["gts_sb"],
            )
            stageC(1)
            stageD_mm(0)
            stageD_mm(1)
            stageD_tp(0)
            stageD_tp(1)
            stageE(0)
            stageE(1)
            stageF(0)
            stageF(1)

    nc.compile()
    return nc


def _compute_col_fast(m1, m2, sm):
    """Exact col == ones proof via a cheap sufficient condition, else None."""
    if m1.min() < 0.0 or m2.min() < 0.0 or sm.min() < 0.0:
        return None
    spos = (sm > 0).astype(F32)
    colnz = np.zeros(N, dtype=bool)
    nz1max = 0.0
    nz2max = 0.0
    for b in range(B):
        p1 = (m1[b] > 0).astype(F32)
        p2 = (m2[b] > 0).astype(F32)
        nz1max = max(nz1max, float((p1 @ spos[b]).max()))
        nz2max = max(nz2max, float((p2 @ spos[b]).max()))
        colnz |= ((p1 + p2).max(axis=0) > 0) & (spos[b] > 0)
    if nz1max <= CHILDS // 4 and nz2max <= CHILDS // 2 and colnz.all():
        return np.ones(N, dtype=F32)
    return None


def _compute_col_slow(m1, m2, sm, li, lj):
    """Exact replica of the reference top-k column-union (numpy)."""
    k4, k2 = CHILDS // 4, CHILDS // 2
    col = np.zeros(N, dtype=bool)
    for b in range(B):
        logits = li[b][:, None] + lj[b][None, :]
        a = 1.0 / (1.0 + np.exp(-logits.astype(F32)))
        mr1 = m1[b] * sm[b][None, :]
        mr2 = m2[b] * sm[b][None, :]
        a1 = a * mr1
        a2 = a * mr2
        # lax.top_k ties -> lowest index; stable argsort on (-a) reproduces it.
        col[np.argsort(-a1, axis=1, kind="stable")[:, :k4].ravel()] = True
        col[np.argsort(a1, axis=1, kind="stable")[:, :k4].ravel()] = True
        col[np.argsort(-a2, axis=1, kind="stable")[:, :k2].ravel()] = True
        col[np.argsort(a2, axis=1, kind="stable")[:, :k4].ravel()] = True
    return col.astype(F32)


def kernel(**inputs):
    x = np.ascontiguousarray(np.asarray(inputs["x"], dtype=F32))
    m1 = np.asarray(inputs["masks_roi1"], dtype=F32)
    m2 = np.asarray(inputs["masks_roi2"], dtype=F32)
    sm = np.asarray(inputs["score_mask"], dtype=F32)
    gt = np.asarray(inputs["gt_feat"], dtype=F32)
    W_att = np.asarray(inputs["W_att"], dtype=F32)
    b_att = np.asarray(inputs["b_att"], dtype=F32)
    W1 = np.asarray(inputs["W1"], dtype=F32)
    b1 = np.asarray(inputs["b1"], dtype=F32)
    W2 = np.asarray(inputs["W2"], dtype=F32)
    b2 = np.asarray(inputs["b2"], dtype=F32)
    g1 = np.asarray(inputs["g1"], dtype=F32)
    beta1 = np.asarray(inputs["beta1"], dtype=F32)
    g2 = np.asarray(inputs["g2"], dtype=F32)
    beta2 = np.asarray(inputs["beta2"], dtype=F32)
    Wg = np.asarray(inputs["Wg"], dtype=F32)
    bg = np.asarray(inputs["bg"], dtype=F32)

    assert x.shape == (B, N, CIN) and W_att.shape == (2 * CIN, 1)

    # ---- host prep: tiny vector math + layout/dtype staging ----
    lj = x.reshape(B * N, CIN) @ W_att[:CIN, 0]
    lj = lj.reshape(B, N)
    li = x.reshape(B * N, CIN) @ W_att[CIN:, 0]
    li = li.reshape(B, N) + b_att[0]

    col = _compute_col_fast(m1, m2, sm)
    if col is None:
        col = _compute_col_slow(m1, m2, sm, li, lj)

    colj = sm * col[None, :]  # [B, N] multiplier along j
    m12c = (m1 + m2) * colj[:, None, :]  # [B, N(i), N(j)]
    # fold f_diag: device computes At = m12c^T(j,i) * sigmoid(z[j,i]); putting
    # fd/sigmoid(z_ii) on the diagonal yields exactly +fd there (the masked
    # diag is 0 whenever fd==1 because score_mask[i]==0 zeroes column i).
    zii = li + lj  # z[i,i] = li[i] + lj[i]
    sii = 1.0 / (1.0 + np.exp(-zii))
    fd = (sm == 0).astype(F32)
    diagval = np.minimum(fd / np.maximum(sii, 1e-6), 440.0)
    ii = np.arange(N)
    m12c[:, ii, ii] += diagval
    m12cT = np.ascontiguousarray(m12c.transpose(0, 2, 1)).astype(F8)
    gxT = np.ascontiguousarray(
        np.concatenate([gt.transpose(0, 2, 1), x.transpose(0, 2, 1)], axis=1)
    ).astype(F16)
    lirow = li.astype(F16)
    ljT = np.ascontiguousarray(lj.reshape(B, NT, 128).transpose(0, 2, 1)).astype(F32)

    # Weights: packed per-chunk transposed layouts for the grouped convs.
    w1bd = np.zeros((CIN, MID), dtype=F32)
    for g in range(G):
        w1bd[64 * g : 64 * (g + 1), 128 * g : 128 * (g + 1)] = W1[
            128 * g : 128 * (g + 1), :
        ].T
    w1K = [w1bd[128 * cc : 128 * cc + 128, 256 * cc : 256 * cc + 256] for cc in range(2)]
    w2K = [W2[64 * g : 64 * (g + 1), :].T for g in range(G)]

    blk = np.concatenate(
        [np.eye(128, dtype=F32)]
        + [Wg[128 * cc : 128 * (cc + 1), :] for cc in range(2)]
        + [w1K[cc] for cc in range(2)]
        + [w2K[mc] for mc in range(4)],
        axis=1,
    ).astype(F16)
    crow = np.concatenate(
        [np.ones(128, dtype=F32), bg, b1, b2]
    ).reshape(1, 1152).astype(F16)
    shared = {
        "constblk": np.ascontiguousarray(blk),
        "constrow": np.ascontiguousarray(crow),
        "g1row": g1.reshape(1, MID).astype(ml_dtypes.bfloat16),
        "beta1row": beta1.reshape(1, MID).astype(F32),
    }
    in_maps = []
    for c in range(NCORES):
        s = slice(B_LOC * c, B_LOC * (c + 1))
        in_maps.append(
            {
                "m12cT": m12cT[s],
                "gxT": gxT[s],
                "lirow": lirow[s],
                "ljT": ljT[s],
                **shared,
            }
        )

    beta_key = bool(np.any(beta1))
    if beta_key not in _PROGRAM_CACHE:
        _PROGRAM_CACHE[beta_key] = _build_program(beta_key)
    nc = _PROGRAM_CACHE[beta_key]

    global _LAST_IN_MAPS
    _LAST_IN_MAPS = in_maps

    from concourse.bass_utils import run_bass_kernel_spmd

    res = run_bass_kernel_spmd(nc, in_maps, core_ids=list(range(NCORES)))
    results = res.results if hasattr(res, "results") else res

    gts = np.concatenate([results[c]["gts"] for c in range(NCORES)], axis=0)
    xhat2 = np.concatenate([results[c]["node"] for c in range(NCORES)], axis=0)
    o2t = np.concatenate([results[c]["o2t"] for c in range(NCORES)], axis=0)
    gts = gts.astype(F32)
    node_feat = xhat2.astype(F32) * g2[None, None, :] + beta2[None, None, :]
    output2 = o2t.astype(F32) + node_feat
    return output2, gts, node_feat


# revision 39
# speedup vs baseline: 1.2785x; 1.0239x over previous
"""Trainium2 Bass kernel for nn_Graph_module_net_0_loss_type_18631568130084.

GNN message-passing block:
  gts       = relu(gt_feat @ Wg + bg)
  attn[i,j] = sigmoid(x[j]@Wq + x[i]@Wk + b_att)          (H == 1)
  atten     = (attn * (mr1+mr2) * col + f_diag) / CHILDS  ([B,H,Nj,Ni])
  o1 = relu(gconv1(x^T)); o1 += ln1(o1 @ atten)^T
  o2 = relu(gconv2(o1));  node_feat = ln2(o2 @ atten);  output2 = (o2 + node_feat^T)^T

Sharding: data-parallel over batch B=16 -> 2 batches per core on 8 cores.

v3 design notes (on top of the v1 transposed layout):
 * Masks pre-combined on host: m12c = (m1+m2) * (score*col)[j] in fp8e4
   ({0,1,2} and the zeros are exact).  The f_diag diagonal is folded in as
   fd[i]/sigmoid(z_ii) (exact where score==0 because the mask diag is 0
   there), so atten^T = m12c ⊙ sigmoid(z) with no device-side fixups.
 * The two big [N,N]x[N,M] contractions (stages D/F) run as fp8e4 DoubleRow
   matmuls (2 j-chunks per instruction).
 * fp16->fp8 casts of o1/o2 and the o1 residual add ride the DMA engines
   (SWDGE cast-DMA / accum-add DMA), not the compute engines.
 * ln2's gamma/beta are applied on the HOST (node_feat = xhat2*g2+b2 and
   output2 = o2 + node_feat), so stage F only standardizes.
 * Grouped convs use packed per-chunk weights (no block-diag zero columns).
 * Inputs/outputs move as one large DMA per tensor per batch.
 * The global 1/CHILDS scale cancels inside both layernorms, so it is dropped
   and eps is rescaled by CHILDS^2 to keep the math exactly equivalent.
 * The top-k "col" mask is computed exactly on the host: a cheap sufficient
   condition proves col == all-ones; otherwise an exact numpy replica runs.
"""

import numpy as np
import ml_dtypes

B = 16
N = 1024
CIN = 256
MID = 512
OUT = 256
G = 4
CHILDS = 512
NCORES = 8
B_LOC = B // NCORES  # 2
NT = N // 128  # 8
EPS_LN = 1e-6 * float(CHILDS) ** 2  # eps rescaled because we drop the 1/CHILDS

F16 = np.float16
F32 = np.float32
F8 = ml_dtypes.float8_e4m3

_PROGRAM_CACHE = {}


def _build_program(beta1_nz: bool):
    import concourse.bacc as bacc
    import concourse.bass as bass
    import concourse.tile as tile
    from concourse import mybir

    f8 = mybir.dt.float8e4
    f16 = mybir.dt.float16
    bf16 = mybir.dt.bfloat16
    f32 = mybir.dt.float32
    AF = mybir.ActivationFunctionType
    OP = mybir.AluOpType
    DR = mybir.MatmulPerfMode.DoubleRow

    nc = bacc.Bacc("TRN2", debug=False)

    def din(name, shape, dt):
        return nc.dram_tensor(name, shape, dt, kind="ExternalInput").ap()

    def dout(name, shape, dt):
        return nc.dram_tensor(name, shape, dt, kind="ExternalOutput").ap()

    # Per-core inputs (leading dim B_LOC where batch-dependent).
    m12_d = din("m12cT", [B_LOC, N, N], f8)       # (m1+m2)*(score*col) + diag, ^T
    gxT_d = din("gxT", [B_LOC, 2 * CIN, N], f16)  # [gt^T ; x^T]  [c, n]
    lirow_d = din("lirow", [B_LOC, N], f16)       # x@Wk + b_att      (per-i row)
    ljT_d = din("ljT", [B_LOC, 128, NT], f32)     # x@Wq chunked      (per-j bias)
    # Replicated weights, packed: blk = [ident | wg 2x256 | w1 2x256 | w2 4x64],
    # crow = [ones 128 | bg 256 | b1 512 | b2 256].
    blk_d = din("constblk", [128, 1408], f16)
    crow_d = din("constrow", [1, 1152], f16)
    g1_d = din("g1row", [1, MID], bf16)
    beta1_d = din("beta1row", [1, MID], f32)

    gts_d = dout("gts", [B_LOC, N, OUT], f16)
    node_d = dout("node", [B_LOC, N, OUT], f16)   # standardized xhat2 (pre-g2)
    o2o_d = dout("o2t", [B_LOC, N, OUT], f16)

    with tile.TileContext(nc) as tc:
        with tc.tile_pool(name="const", bufs=1) as constp, \
             tc.tile_pool(name="big", bufs=2) as bigp, \
             tc.tile_pool(name="work", bufs=4) as workp, \
             tc.tile_pool(name="outs", bufs=3) as outp, \
             tc.tile_pool(name="mm", bufs=6, space="PSUM") as mmp, \
             tc.tile_pool(name="tp", bufs=2, space="PSUM") as tpp:

            # ---- early DMAs: masks on the SWDGE queue, sigmoid inputs on
            # scalar, conv inputs + packed consts on sync ----
            # tiny sigmoid inputs FIRST (the big mask DMAs would otherwise
            # head-of-line-block them on the DMA engines)
            lirow_ts, ljT_ts = [], []
            for b in range(B_LOC):
                lirow_t = workp.tile([128, N], f16, tag="lirow")
                nc.scalar.dma_start(
                    out=lirow_t, in_=lirow_d[b : b + 1, :].to_broadcast([128, N])
                )
                lirow_ts.append(lirow_t)
                ljT_t = workp.tile([128, NT], f32, tag="ljT")
                nc.scalar.dma_start(out=ljT_t, in_=ljT_d[b])
                ljT_ts.append(ljT_t)
            m12b_t = bigp.tile([128, B_LOC, NT, N], f8, name="m12b", tag="m12b")
            for b in range(B_LOC):
                for h in range(2):
                    nc.gpsimd.dma_start(
                        out=m12b_t[:, b, 4 * h : 4 * h + 4, :],
                        in_=m12_d[
                            b, 512 * h : 512 * h + 512, :
                        ].rearrange("(t p) n -> p t n", p=128),
                    )
            m12_ts = [m12b_t[:, 0], m12b_t[:, 1]]

            crow_t = constp.tile([1, 1152], f16)
            nc.sync.dma_start(out=crow_t, in_=crow_d)
            ones_t = crow_t[:, 0:128]
            bg_t = crow_t[:, 128:384]
            b1_t = crow_t[:, 384:896]
            b2_t = crow_t[:, 896:1152]
            gx_t0 = bigp.tile([128, 4, N], f16, name="gx0", tag="gxT")
            nc.sync.dma_start(
                out=gx_t0, in_=gxT_d[0].rearrange("(c p) n -> p c n", p=128)
            )
            blk_t = constp.tile([128, 1408], f16)
            nc.sync.dma_start(out=blk_t, in_=blk_d)
            ident_t = blk_t[:, 0:128]

            def wg_k(cc):
                return blk_t[:, 128 + cc * 256 : 128 + (cc + 1) * 256]

            def w1_k(cc):
                return blk_t[:, 640 + cc * 256 : 640 + (cc + 1) * 256]

            def w2_k(mc):
                return blk_t[:, 1152 + mc * 64 : 1152 + (mc + 1) * 64]

            gx_t1 = bigp.tile([128, 4, N], f16, name="gx1", tag="gxT")
            nc.sync.dma_start(
                out=gx_t1, in_=gxT_d[1].rearrange("(c p) n -> p c n", p=128)
            )
            gtT_ts = [gx_t0[:, 0:2], gx_t1[:, 0:2]]
            xT_ts = [gx_t0[:, 2:4], gx_t1[:, 2:4]]
            g1row_t = constp.tile([128, MID], bf16)
            nc.sync.dma_start(out=g1row_t, in_=g1_d.to_broadcast([128, MID]))
            if beta1_nz:
                beta1_t = constp.tile([128, MID], f32)
                nc.sync.dma_start(out=beta1_t, in_=beta1_d.to_broadcast([128, MID]))
            eps_t = constp.tile([128, 1], f32)
            nc.vector.memset(eps_t, EPS_LN)
            warm_t = constp.tile([128, 1], f16)
            nc.scalar.activation(out=warm_t, in_=eps_t, func=AF.Sigmoid)

            # ---- per-batch tile sets ----
            st = []
            for b in range(B_LOC):
                st.append({
                    "At8": bigp.tile([128, NT, N], f8, name="At8", tag="At8"),
                    "o1t": bigp.tile([128, NT, MID], f16, name="o1t", tag="o1t"),
                    "o1t8": bigp.tile([128, NT, MID], f8, name="o1t8", tag="o1t8"),
                    "o1nT": bigp.tile([128, NT, MID], f16, name="o1nT", tag="o1nT"),
                    "o1n": bigp.tile([128, 4, N], f16, name="o1n", tag="o1n"),
                    "o2t": bigp.tile([128, NT, OUT], f16, name="o2t", tag="o2t"),
                    "o2t8": bigp.tile([128, NT, OUT], f8, name="o2t8", tag="o2t8"),
                    "gts_sb": bigp.tile([128, NT, OUT], f16, name="gts_sb", tag="gts_sb"),
                    "node_sb": bigp.tile([128, NT, OUT], f16, name="node_sb", tag="node_sb"),
                })

            def stageA_jt(b, jt):
                At8 = st[b]["At8"]
                sg = workp.tile([128, N], f16, tag="sg")
                nc.scalar.activation(
                    out=sg, in_=lirow_ts[b], func=AF.Sigmoid,
                    bias=ljT_ts[b][:, jt : jt + 1], scale=1.0,
                )
                nc.vector.tensor_tensor(
                    At8[:, jt, :], m12_ts[b][:, jt, :], sg, op=OP.mult
                )

            def stageB_nt(b, nt):
                gts_sb = st[b]["gts_sb"]
                ps = mmp.tile([128, MID], f32, tag="ps")
                p256 = ps[:, :OUT]
                nc.tensor.matmul(p256, lhsT=ones_t, rhs=bg_t, start=True, stop=False)
                for cc in range(2):
                    nc.tensor.matmul(
                        p256,
                        lhsT=gtT_ts[b][:, cc, nt * 128 : (nt + 1) * 128],
                        rhs=wg_k(cc),
                        start=False, stop=(cc == 1),
                    )
                nc.vector.tensor_scalar_max(gts_sb[:, nt, :], p256, 0.0)

            def stageC_jt(b, jt):
                o1t, o1t8 = st[b]["o1t"], st[b]["o1t8"]
                ps = mmp.tile([128, MID], f32, tag="ps")
                nc.tensor.matmul(ps, lhsT=ones_t, rhs=b1_t, start=True, stop=False)
                for cc in range(2):
                    nc.tensor.matmul(
                        ps[:, cc * 256 : (cc + 1) * 256],
                        lhsT=xT_ts[b][:, cc, jt * 128 : (jt + 1) * 128],
                        rhs=w1_k(cc),
                        start=False, stop=(cc == 1),
                        skip_group_check=True,
                    )
                nc.scalar.activation(out=o1t[:, jt, :], in_=ps, func=AF.Relu)
                nc.gpsimd.dma_start(out=o1t8[:, jt, :], in_=o1t[:, jt, :])

            def stageB(b):
                for nt in range(NT):
                    stageB_nt(b, nt)
                nc.sync.dma_start(
                    out=gts_d[b].rearrange("(t p) f -> p t f", p=128),
                    in_=st[b]["gts_sb"],
                )

            def stageC(b):
                for jt in range(NT):
                    stageC_jt(b, jt)

            def stageD_mm(b):
                At8 = st[b]["At8"]
                o1t, o1t8 = st[b]["o1t"], st[b]["o1t8"]
                o1nT = st[b]["o1nT"]
                for it in range(NT):
                    ps = mmp.tile([128, MID], f32, tag="ps")
                    for jc in range(NT // 2):
                        nc.tensor.matmul(
                            ps,
                            lhsT=At8[:, 2 * jc : 2 * jc + 2, it * 128 : (it + 1) * 128],
                            rhs=o1t8[:, 2 * jc : 2 * jc + 2, :],
                            start=(jc == 0), stop=(jc == NT // 2 - 1),
                            perf_mode=DR,
                        )
                    sv = workp.tile([128, 6], f32, tag="sv")
                    nc.vector.bn_stats(out=sv, in_=ps)
                    mv = workp.tile([128, 2], f32, tag="mv")
                    nc.vector.bn_aggr(out=mv, in_=sv)
                    std = workp.tile([128, 1], f32, tag="std")
                    nc.scalar.activation(
                        out=std, in_=mv[:, 1:2], func=AF.Sqrt, bias=eps_t
                    )
                    rstd = workp.tile([128, 1], f32, tag="rstd")
                    nc.vector.reciprocal(out=rstd, in_=std)
                    outer = workp.tile([128, MID], bf16, tag="outer")
                    nc.vector.tensor_scalar_mul(outer, g1row_t, rstd)
                    nc.vector.scalar_tensor_tensor(
                        out=o1nT[:, it, :], in0=ps, scalar=mv[:, 0:1], in1=outer,
                        op0=OP.subtract, op1=OP.mult,
                    )
                    if beta1_nz:
                        nc.vector.tensor_tensor(
                            o1nT[:, it, :], o1nT[:, it, :], beta1_t, op=OP.add
                        )
                    # residual add per chunk on the DMA engines: o1nT += o1t
                    nc.gpsimd.dma_start(
                        out=o1nT[:, it, :], in_=o1t[:, it, :], accum_op=OP.add
                    )
            def stageD_tp(b):
                o1nT, o1n = st[b]["o1nT"], st[b]["o1n"]
                # transposes -> o1n [m, j]
                for it in range(NT):
                    tp = tpp.tile([128, 4, 128], f16, tag="tp")
                    for mc in range(4):
                        nc.tensor.transpose(
                            tp[:, mc, :], o1nT[:, it, mc * 128 : (mc + 1) * 128],
                            ident_t,
                        )
                    nc.scalar.activation(
                        out=o1n[:, :, it * 128 : (it + 1) * 128], in_=tp,
                        func=AF.Copy,
                    )

            def stageE(b):
                o1n, o2t, o2t8 = st[b]["o1n"], st[b]["o2t"], st[b]["o2t8"]
                for jt in range(NT):
                    ps = mmp.tile([128, MID], f32, tag="ps")
                    p256 = ps[:, :OUT]
                    nc.tensor.matmul(p256, lhsT=ones_t, rhs=b2_t, start=True, stop=False)
                    for mc in range(4):
                        nc.tensor.matmul(
                            p256[:, mc * 64 : (mc + 1) * 64],
                            lhsT=o1n[:, mc, jt * 128 : (jt + 1) * 128],
                            rhs=w2_k(mc),
                            start=False, stop=(mc == 3),
                            skip_group_check=True,
                        )
                    nc.scalar.activation(out=o2t[:, jt, :], in_=p256, func=AF.Relu)
                    nc.gpsimd.dma_start(out=o2t8[:, jt, :], in_=o2t[:, jt, :])
                nc.sync.dma_start(
                    out=o2o_d[b].rearrange("(t p) f -> p t f", p=128), in_=o2t
                )

            def stageF(b):
                At8, o2t8 = st[b]["At8"], st[b]["o2t8"]
                node_sb = st[b]["node_sb"]
                for it in range(NT):
                    ps = mmp.tile([128, MID], f32, tag="ps")
                    p256 = ps[:, :OUT]
                    for jc in range(NT // 2):
                        nc.tensor.matmul(
                            p256,
                            lhsT=At8[:, 2 * jc : 2 * jc + 2, it * 128 : (it + 1) * 128],
                            rhs=o2t8[:, 2 * jc : 2 * jc + 2, :],
                            start=(jc == 0), stop=(jc == NT // 2 - 1),
                            perf_mode=DR,
                        )
                    sv = workp.tile([128, 6], f32, tag="sv")
                    nc.vector.bn_stats(out=sv, in_=p256)
                    mv = workp.tile([128, 2], f32, tag="mv")
                    nc.vector.bn_aggr(out=mv, in_=sv)
                    std = workp.tile([128, 1], f32, tag="std")
                    nc.scalar.activation(
                        out=std, in_=mv[:, 1:2], func=AF.Sqrt, bias=eps_t
                    )
                    rstd = workp.tile([128, 1], f32, tag="rstd")
                    nc.vector.reciprocal(out=rstd, in_=std)
                    # nf = (ps - mu) * rstd  via ACT: scale=rstd, bias=-mu*rstd
                    nmu = workp.tile([128, 1], f32, tag="nmu")
                    nc.vector.tensor_tensor(nmu, mv[:, 0:1], rstd, op=OP.mult)
                    nmun = workp.tile([128, 1], f32, tag="nmun")
                    nc.vector.tensor_scalar_mul(nmun, nmu, -1.0)
                    nc.scalar.activation(
                        out=node_sb[:, it, :], in_=p256, func=AF.Identity,
                        bias=nmun, scale=rstd,
                    )
                    if it == NT // 2 - 1:
                        nc.sync.dma_start(
                            out=node_d[b, : N // 2].rearrange(
                                "(t p) f -> p t f", p=128
                            ),
                            in_=node_sb[:, : NT // 2, :],
                        )
                nc.sync.dma_start(
                    out=node_d[b, N // 2 :].rearrange("(t p) f -> p t f", p=128),
                    in_=node_sb[:, NT // 2 :, :],
                )

            # schedule: A first (sigmoid table residency); batch-1 conv work
            # fills the PE while batch-0's casts/LN drain, and vice versa.
            # phase 0: both batches' sigmoids and all of B/C interleaved.
            # sigmoid+relu share one ACT table; B-relu runs on DVE, C-relu on
            # ACT, so PSUM recycles at PE rate while the A-chain streams.
            for jt in range(NT):
                stageA_jt(0, jt)
                stageB_nt(0, jt)
                stageC_jt(0, jt)
                stageA_jt(1, jt)
                stageB_nt(1, jt)
            nc.sync.dma_start(
                out=gts_d[0].rearrange("(t p) f -> p t f", p=128),
                in_=st[0]["gts_sb"],
            )
            nc.sync.dma_start(
                out=gts_d[1].rearrange("(t p) f -> p t f", p=128),
                in_=st[1]["gts_sb"],
            )
            stageC(1)
            stageD_mm(0)
            stageD_mm(1)
            stageD_tp(0)
            stageD_tp(1)
            stageE(0)
            stageE(1)
            stageF(0)
            stageF(1)

    nc.compile()
    return nc


def _compute_col_fast(m1, m2, sm):
    """Exact col == ones proof via a cheap sufficient condition, else None."""
    if m1.min() < 0.0 or m2.min() < 0.0 or sm.min() < 0.0:
        return None
    spos = (sm > 0).astype(F32)
    colnz = np.zeros(N, dtype=bool)
    nz1max = 0.0
    nz2max = 0.0
    for b in range(B):
        p1 = (m1[b] > 0).astype(F32)
        p2 = (m2[b] > 0).astype(F32)
        nz1max = max(nz1max, float((p1 @ spos[b]).max()))
        nz2max = max(nz2max, float((p2 @ spos[b]).max()))
        colnz |= ((p1 + p2).max(axis=0) > 0) & (spos[b] > 0)
    if nz1max <= CHILDS // 4 and nz2max <= CHILDS // 2 and colnz.all():
        return np.ones(N, dtype=F32)
    return None


def _compute_col_slow(m1, m2, sm, li, lj):
    """Exact replica of the reference top-k column-union (numpy)."""
    k4, k2 = CHILDS // 4, CHILDS // 2
    col = np.zeros(N, dtype=bool)
    for b in range(B):
        logits = li[b][:, None] + lj[b][None, :]
        a = 1.0 / (1.0 + np.exp(-logits.astype(F32)))
        mr1 = m1[b] * sm[b][None, :]
        mr2 = m2[b] * sm[b][None, :]
        a1 = a * mr1
        a2 = a * mr2
        # lax.top_k ties -> lowest index; stable argsort on (-a) reproduces it.
        col[np.argsort(-a1, axis=1, kind="stable")[:, :k4].ravel()] = True
        col[np.argsort(a1, axis=1, kind="stable")[:, :k4].ravel()] = True
        col[np.argsort(-a2, axis=1, kind="stable")[:, :k2].ravel()] = True
        col[np.argsort(a2, axis=1, kind="stable")[:, :k4].ravel()] = True
    return col.astype(F32)


def kernel(**inputs):
    x = np.ascontiguousarray(np.asarray(inputs["x"], dtype=F32))
    m1 = np.asarray(inputs["masks_roi1"], dtype=F32)
    m2 = np.asarray(inputs["masks_roi2"], dtype=F32)
    sm = np.asarray(inputs["score_mask"], dtype=F32)
    gt = np.asarray(inputs["gt_feat"], dtype=F32)
    W_att = np.asarray(inputs["W_att"], dtype=F32)
    b_att = np.asarray(inputs["b_att"], dtype=F32)
    W1 = np.asarray(inputs["W1"], dtype=F32)
    b1 = np.asarray(inputs["b1"], dtype=F32)
    W2 = np.asarray(inputs["W2"], dtype=F32)
    b2 = np.asarray(inputs["b2"], dtype=F32)
    g1 = np.asarray(inputs["g1"], dtype=F32)
    beta1 = np.asarray(inputs["beta1"], dtype=F32)
    g2 = np.asarray(inputs["g2"], dtype=F32)
    beta2 = np.asarray(inputs["beta2"], dtype=F32)
    Wg = np.asarray(inputs["Wg"], dtype=F32)
    bg = np.asarray(inputs["bg"], dtype=F32)

    assert x.shape == (B, N, CIN) and W_att.shape == (2 * CIN, 1)

    # ---- host prep: tiny vector math + layout/dtype staging ----
    lj = x.reshape(B * N, CIN) @ W_att[:CIN, 0]
    lj = lj.reshape(B, N)
    li = x.reshape(B * N, CIN) @ W_att[CIN:, 0]
    li = li.reshape(B, N) + b_att[0]

    col = _compute_col_fast(m1, m2, sm)
    if col is None:
        col = _compute_col_slow(m1, m2, sm, li, lj)

    colj = sm * col[None, :]  # [B, N] multiplier along j
    m12c = (m1 + m2) * colj[:, None, :]  # [B, N(i), N(j)]
    # fold f_diag: device computes At = m12c^T(j,i) * sigmoid(z[j,i]); putting
    # fd/sigmoid(z_ii) on the diagonal yields exactly +fd there (the masked
    # diag is 0 whenever fd==1 because score_mask[i]==0 zeroes column i).
    zii = li + lj  # z[i,i] = li[i] + lj[i]
    sii = 1.0 / (1.0 + np.exp(-zii))
    fd = (sm == 0).astype(F32)
    diagval = np.minimum(fd / np.maximum(sii, 1e-6), 440.0)
    ii = np.arange(N)
    m12c[:, ii, ii] += diagval
    m12cT = np.ascontiguousarray(m12c.transpose(0, 2, 1)).astype(F8)
    gxT = np.ascontiguousarray(
        np.concatenate([gt.transpose(0, 2, 1), x.transpose(0, 2, 1)], axis=1)
    ).astype(F16)
    lirow = li.astype(F16)
    ljT = np.ascontiguousarray(lj.reshape(B, NT, 128).transpose(0, 2, 1)).astype(F32)

    # Weights: packed per-chunk transposed layouts for the grouped convs.
    w1bd = np.zeros((CIN, MID), dtype=F32)
    for g in range(G):
        w1bd[64 * g : 64 * (g + 1), 128 * g : 128 * (g + 1)] = W1[
            128 * g : 128 * (g + 1), :
        ].T
    w1K = [w1bd[128 * cc : 128 * cc + 128, 256 * cc : 256 * cc + 256] for cc in range(2)]
    w2K = [W2[64 * g : 64 * (g + 1), :].T for g in range(G)]

    blk = np.concatenate(
        [np.eye(128, dtype=F32)]
        + [Wg[128 * cc : 128 * (cc + 1), :] for cc in range(2)]
        + [w1K[cc] for cc in range(2)]
        + [w2K[mc] for mc in range(4)],
        axis=1,
    ).astype(F16)
    crow = np.concatenate(
        [np.ones(128, dtype=F32), bg, b1, b2]
    ).reshape(1, 1152).astype(F16)
    shared = {
        "constblk": np.ascontiguousarray(blk),
        "constrow": np.ascontiguousarray(crow),
        "g1row": g1.reshape(1, MID).astype(ml_dtypes.bfloat16),
        "beta1row": beta1.reshape(1, MID).astype(F32),
    }
    in_maps = []
    for c in range(NCORES):
        s = slice(B_LOC * c, B_LOC * (c + 1))
        in_maps.append(
            {
                "m12cT": m12cT[s],
                "gxT": gxT[s],
                "lirow": lirow[s],
                "ljT": ljT[s],
                **shared,
            }
        )

    beta_key = bool(np.any(beta1))
    if beta_key not in _PROGRAM_CACHE:
        _PROGRAM_CACHE[beta_key] = _build_program(beta_key)
    nc = _PROGRAM_CACHE[beta_key]

    global _LAST_IN_MAPS
    _LAST_IN_MAPS = in_maps

    from concourse.bass_utils import run_bass_kernel_spmd

    res = run_bass_kernel_spmd(nc, in_maps, core_ids=list(range(NCORES)))
    results = res.results if hasattr(res, "results") else res

    gts = np.concatenate([results[c]["gts"] for c in range(NCORES)], axis=0)
    xhat2 = np.concatenate([results[c]["node"] for c in range(NCORES)], axis=0)
    o2t = np.concatenate([results[c]["o2t"] for c in range(NCORES)], axis=0)
    gts = gts.astype(F32)
    node_feat = xhat2.astype(F32) * g2[None, None, :] + beta2[None, None, :]
    output2 = o2t.astype(F32) + node_feat
    return output2, gts, node_feat
